# revision 43
# baseline (speedup 1.0000x reference)
"""Multi-head causal attention (B=4, S=2048, E=1024, H=16, D=64) on 8 trn2 cores.

Sharding: core c handles batch b = c//2 and head-group g = c%2 (8 heads each).
Each core computes its partial output projection over its 512 local concat
columns; the host sums the two partials per batch and adds bp.

Layout strategy (per core):
  - x is pre-transposed on host: xT [1024(+pad), S] as [128, 8, S] e-tiles.
  - Q^T, K^T computed as [d, s] (d on partitions, 2 heads per 128-partition
    pair tile) so scores come out transposed: scoresT [t, s].  The two heads
    of a pair sit at partitions 0-63 / 64-127, so the score matmuls of a pair
    run concurrently on different PE row groups.
  - V kept natural [t, d] with a ones column per head (66-col stride), so the
    PV matmul also produces the softmax denominator as row 64/65 of its
    output.  V bias is added on DVE from a host-broadcast tile.
  - Softmax: exp on ACT (no max subtraction -- scores are O(1) by
    construction), causal masking via ONE broadcast multiplicative 0/1 mask
    on DVE per diag item, both denominators merged to rows 64/65 with a
    single DVE add, broadcast across partitions via a K=2 matmul,
    reciprocal on DVE, normalize into concat^T, output projection from
    concat^T, y output in bf16 (host does f32 sum).
  - Attention is software-pipelined: the PV matmuls of item ti are emitted
    after the score matmuls of item ti+2, so the PE FIFO never head-blocks
    on the ACT exp of the current item (exp latency ~1.15us/item is the
    pacer in late chunks; the 2-item lag lets scores+fillers run under it).
  - Scheduling: attention items are interleaved with "filler" pieces (2
    matmuls each) from the QKV projections of later chunks and the output
    projection of earlier chunks, spread by estimated cost so the tensor
    engine never starves while ACT/DVE work through the softmax chain.
  - Head: first DMAs are split into 256KB et-pair slices across both HWDGE
    queues so the first Q matmuls start ~9us instead of ~14us; the 2MB wp
    (not needed until chunk 2) is deferred to the queue tails; y outputs go
    out on the HWDGE queues (idle after input load) instead of SWDGE.
"""

import numpy as np

B, S, E, H, D = 4, 2048, 1024, 16, 64
NCORES = 8
PAIRS = 4  # head pairs per core (8 heads)
ET = 8  # e-tiles of 128 for the contraction over E
SCH = 4  # s-chunks of 512
VW = 66  # V columns per head: 64 d + 1 ones + 1 pad
SCALE = float(D) ** -0.5

MM_DTYPE = "bfloat16"

_CACHE = {}


def host_round(a):
    import ml_dtypes

    return np.ascontiguousarray(a, np.float32).astype(ml_dtypes.bfloat16)


def _build():
    import concourse.tile as tile
    from concourse import bacc, mybir
    from contextlib import ExitStack

    f32 = mybir.dt.float32
    bf16 = mybir.dt.bfloat16
    mdt = getattr(mybir.dt, MM_DTYPE)
    AF = mybir.ActivationFunctionType

    nc = bacc.Bacc("TRN2", target_bir_lowering=False, debug=False, num_devices=NCORES)

    xt_d = nc.dram_tensor("xt", [SCH, 128, ET, 512], mdt, kind="ExternalInput").ap()
    wq_d = nc.dram_tensor("wq", [128, ET, 512], mdt, kind="ExternalInput").ap()
    wk_d = nc.dram_tensor("wk", [128, ET, 512], mdt, kind="ExternalInput").ap()
    wv_d = nc.dram_tensor("wv", [128, ET, 512], mdt, kind="ExternalInput").ap()
    wp_d = nc.dram_tensor("wp", [128, PAIRS, E], mdt, kind="ExternalInput").ap()
    bq_d = nc.dram_tensor("bq", [128, PAIRS], f32, kind="ExternalInput").ap()
    bk_d = nc.dram_tensor("bk", [128, PAIRS], f32, kind="ExternalInput").ap()
    bvb_d = nc.dram_tensor("bvb", [128, 512], f32, kind="ExternalInput").ap()
    mask_d = nc.dram_tensor("mask", [128, 4, 2, 512], mdt, kind="ExternalInput").ap()
    eye2_d = nc.dram_tensor("eye2", [128, 128], mdt, kind="ExternalInput").ap()
    y_d = nc.dram_tensor("y", [S, E], bf16, kind="ExternalOutput").ap()

    with tile.TileContext(nc) as tc, ExitStack() as ctx:
        pers = ctx.enter_context(tc.tile_pool(name="pers", bufs=1))
        work = ctx.enter_context(tc.tile_pool(name="work", bufs=1))
        psp = ctx.enter_context(tc.tile_pool(name="psp", bufs=1, space="PSUM"))

        qt = pers.tile([128, PAIRS, S], mdt)  # Q^T pair tiles
        kt = pers.tile([128, PAIRS, S], mdt)  # K^T pair tiles
        va = pers.tile([128, 16, 8 * VW], mdt)  # V (+ones col) per t-block
        cat = pers.tile([128, PAIRS, S], mdt)  # concat^T
        bq_sb = pers.tile([128, PAIRS], f32)
        bk_sb = pers.tile([128, PAIRS], f32)
        bvb_sb = pers.tile([128, 512], f32)
        eye2 = pers.tile([128, 128], mdt)  # bc lhsT: rows 64/65 block-select
        wq_sb = pers.tile([128, ET, 512], mdt)
        wk_sb = pers.tile([128, ET, 512], mdt)
        wv_sb = pers.tile([128, ET, 512], mdt)
        wp_sb = pers.tile([128, PAIRS, E], mdt)
        mask_sb = pers.tile([128, 4, 2, 512], mdt)  # mask duplicated per head
        y01 = pers.tile([128, 8, 512], f32)  # chunk-3 proj r0+r1 partials
        xts = [
            work.tile([128, ET, 512], mdt, tag=f"xt{j % 2}", name=f"xt{j}")
            for j in range(SCH)
        ]

        # ---- head DMAs: only sync/scalar (HWDGE) + gpsimd (SWDGE) can issue.
        # First-needed data goes first in 256KB et-pair slices alternating
        # across the two HWDGE queues so Q(0) compute can start as soon as
        # the first pair lands.  wp (2MB, needed only from chunk 2) is
        # deferred to the queue tails so it doesn't steal HBM bandwidth from
        # the critical path.  gpsimd (SWDGE) only gets the small tensors.
        nc.sync.dma_start(out=wq_sb[:, 0:2, :], in_=wq_d[:, 0:2, :])
        nc.scalar.dma_start(out=xts[0][:, 0:2, :], in_=xt_d[0][:, 0:2, :])
        nc.sync.dma_start(out=wq_sb[:, 2:4, :], in_=wq_d[:, 2:4, :])
        nc.scalar.dma_start(out=xts[0][:, 2:4, :], in_=xt_d[0][:, 2:4, :])
        nc.sync.dma_start(out=xts[0][:, 4:8, :], in_=xt_d[0][:, 4:8, :])
        nc.scalar.dma_start(out=wq_sb[:, 4:8, :], in_=wq_d[:, 4:8, :])
        nc.sync.dma_start(out=wk_sb[:, 0:4, :], in_=wk_d[:, 0:4, :])
        nc.scalar.dma_start(out=wk_sb[:, 4:8, :], in_=wk_d[:, 4:8, :])
        nc.gpsimd.dma_start(out=eye2, in_=eye2_d)
        nc.gpsimd.dma_start(out=bq_sb, in_=bq_d)
        nc.gpsimd.dma_start(out=bk_sb, in_=bk_d)
        nc.gpsimd.dma_start(out=bvb_sb, in_=bvb_d)
        nc.gpsimd.dma_start(out=mask_sb, in_=mask_d)
        nc.sync.dma_start(out=wv_sb[:, 0:4, :], in_=wv_d[:, 0:4, :])
        nc.scalar.dma_start(out=wv_sb[:, 4:8, :], in_=wv_d[:, 4:8, :])
        nc.sync.dma_start(out=xts[1][:, 0:4, :], in_=xt_d[1][:, 0:4, :])
        nc.scalar.dma_start(out=xts[1][:, 4:8, :], in_=xt_d[1][:, 4:8, :])
        nc.sync.dma_start(out=xts[2][:, 0:4, :], in_=xt_d[2][:, 0:4, :])
        nc.scalar.dma_start(out=xts[2][:, 4:8, :], in_=xt_d[2][:, 4:8, :])
        nc.sync.dma_start(out=xts[3][:, 0:4, :], in_=xt_d[3][:, 0:4, :])
        nc.scalar.dma_start(out=xts[3][:, 4:8, :], in_=xt_d[3][:, 4:8, :])
        nc.sync.dma_start(out=wp_sb[:, 0:2, :], in_=wp_d[:, 0:2, :])
        nc.scalar.dma_start(out=wp_sb[:, 2:4, :], in_=wp_d[:, 2:4, :])

        # ones + pad columns of va are static: even heads carry ones at col
        # 64 (denominator -> PSUM row 64), odd heads at col 65 (-> row 65),
        # so the two denominators land on rows 64/65 of the two PV psums and
        # merge with a single DVE add
        va_hc = va.rearrange("p i (h c) -> p i h c", c=VW)
        va_pp = va.rearrange("p i (g w) -> p (i g) w", w=2 * VW)
        nc.vector.memset(va_pp[:, :, 64:65], 1.0)
        nc.vector.memset(va_pp[:, :, 65:66], 0.0)
        nc.vector.memset(va_pp[:, :, VW + 64 : VW + 65], 0.0)
        nc.vector.memset(va_pp[:, :, VW + 65 : VW + 66], 1.0)

        # ---------------- work-item emitters ----------------
        qk_state = {}

        def emit_qk_quarter(j, r, which, qi):
            """Quarter of a Q/K projection unit: 2 e-tile matmuls; the last
            quarter finishes the accumulation and adds the bias on DVE."""
            w_sb, dst, b_sb = (
                (wq_sb, qt, bq_sb) if which == "q" else (wk_sb, kt, bk_sb)
            )
            key = (j, r, which)
            if qi == 0:
                qk_state[key] = psp.tile(
                    [128, 512], f32, tag="mm512", bufs=2, name="qkps"
                )
            ps = qk_state[key]
            for et in range(2 * qi, 2 * qi + 2):
                nc.tensor.matmul(
                    ps,
                    lhsT=w_sb[:, et, r * 128 : (r + 1) * 128],
                    rhs=xts[j][:, et, :],
                    start=(et == 0),
                    stop=(et == ET - 1),
                )
            if qi == 3:
                del qk_state[key]
                sjl = slice(j * 512, (j + 1) * 512)
                nc.vector.tensor_scalar_add(
                    dst[:, r, sjl], ps, b_sb[:, r : r + 1]
                )

        def emit_v_quarter(j, ii, qi):
            i = 4 * j + ii
            si = slice(ii * 128, (ii + 1) * 128)
            key = ("v", j, ii)
            if qi == 0:
                qk_state[key] = psp.tile(
                    [128, 512], f32, tag="mm512", bufs=2, name="vps"
                )
            ps = qk_state[key]
            for et in range(2 * qi, 2 * qi + 2):
                nc.tensor.matmul(
                    ps,
                    lhsT=xts[j][:, et, si],
                    rhs=wv_sb[:, et, :],
                    start=(et == 0),
                    stop=(et == ET - 1),
                )
            if qi == 3:
                del qk_state[key]
                va_i = va_hc[:, i]
                nc.vector.tensor_tensor(
                    va_i[:, :, 0:64],
                    ps.rearrange("p (h d) -> p h d", d=64),
                    bvb_sb.rearrange("p (h d) -> p h d", d=64),
                    op=mybir.AluOpType.add,
                )

        attn_state = {}
        attn_pr = {}

        def emit_sc(j, r, ti):
            """Score pair for item ti + exp on ACT (+ causal mask on DVE for
            diagonal items).  PV is emitted separately, 2 items later."""
            if ti == 0:
                attn_state[(j, r)] = [
                    psp.tile([VW, 512], f32, tag=f"o{hh}", bufs=1,
                             name=f"outp{hh}")
                    for hh in range(2)
                ]
            tis = slice(ti * 128, (ti + 1) * 128)
            v = max(ti - 4 * j, 0)
            w = 512 - 128 * v
            sjv = slice(j * 512 + 128 * v, (j + 1) * 512)
            scp = psp.tile([128, 2, 512], f32, tag="sc", bufs=2)
            for hh in range(2):
                po = hh * 64
                nc.tensor.matmul(
                    scp[:, hh, 128 * v :],
                    lhsT=kt[po : po + 64, r, tis],
                    rhs=qt[po : po + 64, r, sjv],
                    start=True,
                    stop=True,
                )
            pr = work.tile([128, 2, 512], mdt, tag="pr", bufs=6)
            nc.scalar.activation(
                pr[:, :, 128 * v :], scp[:, :, 128 * v :], AF.Exp, scale=SCALE
            )
            if v or ti == 4 * j:
                nc.vector.tensor_tensor(
                    pr[:, :, 128 * v :], pr[:, :, 128 * v :],
                    mask_sb[:, v, :, 128 * v :],
                    op=mybir.AluOpType.mult,
                )
            attn_pr[(j, r, ti)] = pr

        def emit_pv(j, r, ti):
            nt = 4 * j + 4
            outps = attn_state[(j, r)]
            pr = attn_pr.pop((j, r, ti))
            v = max(ti - 4 * j, 0)
            for hh in range(2):
                h = 2 * r + hh
                nc.tensor.matmul(
                    outps[hh][:, 128 * v :],
                    lhsT=va[:, ti, h * VW : (h + 1) * VW],
                    rhs=pr[:, hh, 128 * v :],
                    start=(ti == 0),
                    stop=(ti == nt - 1),
                )

        def emit_norm33_half(h):
            """Half-norm for the last pair (3,3): columns [256h, 256h+256).
            Half 0 is final after pv(13), so it (and its proj3b units) is
            emitted BEFORE pv(14)/pv(15) -- Tile serializes the psum-bank
            reads against the remaining PV writes, pipelining the tail."""
            outps = attn_state[(3, 3)]
            cs = slice(256 * h, 256 * h + 256)
            sjl = slice(3 * 512 + 256 * h, 3 * 512 + 256 * h + 256)
            osbd = work.tile([128, 512], mdt, tag="osbd", bufs=2)
            nc.vector.tensor_copy(osbd[64:66, cs], outps[1][64:66, cs])
            nc.vector.tensor_copy(osbd[64:65, cs], outps[0][64:65, cs])
            rdp = psp.tile([128, 512], f32, tag="sc", bufs=2, name="bcst")
            nc.tensor.matmul(
                rdp[:, cs],
                lhsT=eye2[64:66, :],
                rhs=osbd[64:66, cs],
                start=True,
                stop=True,
            )
            rd = work.tile([128, 512], f32, tag="rd", bufs=2)
            nc.vector.reciprocal_approx_fast(rd[:, cs], rdp[:, cs])
            nc.vector.tensor_mul(
                cat[0:64, 3, sjl], outps[0][0:64, cs], rd[0:64, cs]
            )
            nc.vector.tensor_mul(
                cat[64:128, 3, sjl], outps[1][0:64, cs], rd[64:128, cs]
            )
            if h == 1:
                attn_state.pop((3, 3))

        norm_state = {}

        def emit_norm_a(j, r):
            """Denominator copies (DVE only) -- emitted early so the bcst
            matmul in emit_norm_b never exposes the DVE latency on PE."""
            outps = attn_state[(j, r)]
            # both denominators to adjacent partitions 64/65, then ONE K=2
            # matmul broadcasts h0's to rows 0-63 and h1's to rows 64-127
            osbd = work.tile([128, 512], mdt, tag="osbd", bufs=2)
            nc.vector.tensor_copy(osbd[64:66, :], outps[1][64:66, :])
            nc.vector.tensor_copy(osbd[64:65, :], outps[0][64:65, :])
            norm_state[(j, r)] = osbd

        def emit_norm_b(j, r):
            outps = attn_state.pop((j, r))
            osbd = norm_state.pop((j, r))
            sjl = slice(j * 512, (j + 1) * 512)
            rdp = psp.tile([128, 512], f32, tag="sc", bufs=2, name="bcst")
            nc.tensor.matmul(
                rdp,
                lhsT=eye2[64:66, :],
                rhs=osbd[64:66, :],
                start=True,
                stop=True,
            )
            # normalize directly from the PV psum (mixed PSUM+SB inputs are
            # exempt from the equal-base-partition rule)
            rd = work.tile([128, 512], f32, tag="rd", bufs=2)
            nc.vector.reciprocal_approx_fast(rd, rdp)
            nc.vector.tensor_mul(
                cat[0:64, r, sjl], outps[0][0:64, :], rd[0:64, :]
            )
            nc.vector.tensor_mul(
                cat[64:128, r, sjl], outps[1][0:64, :], rd[64:128, :]
            )

        def emit_norm(j, r):
            emit_norm_a(j, r)
            emit_norm_b(j, r)

        def emit_proj_piece(j, sb, f, rr, on_act=False):
            """Output-projection piece: contraction pairs rr=(0,1) or (2,3).
            Tail-woven pieces evacuate psum on the (idle) scalar engine so
            the tail's DVE chain is untouched."""
            ss = slice(sb * 128, (sb + 1) * 128)
            sf = slice(f * 512, (f + 1) * 512)
            key = ("p", j, sb, f)
            if rr[0] == 0:
                qk_state[key] = psp.tile(
                    [128, 512], f32, tag="mm512", bufs=2, name="yproj"
                )
            yp = qk_state[key]
            for r in rr:
                nc.tensor.matmul(
                    yp,
                    lhsT=cat[:, r, ss],
                    rhs=wp_sb[:, r, sf],
                    start=(r == 0),
                    stop=(r == PAIRS - 1),
                )
            if rr[-1] == PAIRS - 1:
                del qk_state[key]
                ykey = ("ys", sb)
                if f == 0:
                    qk_state[ykey] = work.tile(
                        [128, 2, 512], mdt, tag="ys", bufs=2, name="yspair"
                    )
                ys = qk_state[ykey]
                if on_act:
                    nc.scalar.copy(ys[:, f, :], yp)
                else:
                    nc.vector.tensor_copy(ys[:, f, :], yp)
                if f == 1:
                    del qk_state[ykey]
                    nc.sync.dma_start(out=y_d[ss, :], in_=ys)

        def emit_proj3_a(u):
            """Chunk-3 proj unit u (sb=12+u//2, f=u%2): r0+r1 into stash."""
            sb, f = 12 + u // 2, u % 2
            ss = slice(sb * 128, (sb + 1) * 128)
            sf = slice(f * 512, (f + 1) * 512)
            yp = psp.tile([128, 512], f32, tag="mm512", bufs=2, name="yp3a")
            for r in (0, 1):
                nc.tensor.matmul(
                    yp,
                    lhsT=cat[:, r, ss],
                    rhs=wp_sb[:, r, sf],
                    start=(r == 0),
                    stop=(r == 1),
                )
            nc.vector.tensor_copy(y01[:, u, :], yp)

        def emit_proj3_c(u):
            """Chunk-3 proj unit u: fold r2 into the stash after norm(3,2)."""
            sb, f = 12 + u // 2, u % 2
            ss = slice(sb * 128, (sb + 1) * 128)
            sf = slice(f * 512, (f + 1) * 512)
            yp = psp.tile([128, 512], f32, tag="mm512", bufs=2, name="yp3c")
            nc.tensor.matmul(
                yp, lhsT=cat[:, 2, ss], rhs=wp_sb[:, 2, sf], start=True,
                stop=True,
            )
            nc.vector.tensor_tensor(
                y01[:, u, :], y01[:, u, :], yp, op=mybir.AluOpType.add
            )

        def emit_proj3_b(u):
            """Chunk-3 proj unit u: r3 only, add stash, emit output."""
            sb, f = 12 + u // 2, u % 2
            ss = slice(sb * 128, (sb + 1) * 128)
            sf = slice(f * 512, (f + 1) * 512)
            yp = psp.tile([128, 512], f32, tag="mm512", bufs=2, name="yp3b")
            nc.tensor.matmul(
                yp, lhsT=cat[:, 3, ss], rhs=wp_sb[:, 3, sf], start=True,
                stop=True,
            )
            ykey = ("ys3", sb)
            if f == 0:
                qk_state[ykey] = work.tile(
                    [128, 2, 512], mdt, tag="ys", bufs=2, name="yspair3"
                )
            ys = qk_state[ykey]
            nc.vector.tensor_tensor(
                ys[:, f, :], yp, y01[:, u, :], op=mybir.AluOpType.add
            )
            if f == 1:
                del qk_state[ykey]
                nc.sync.dma_start(out=y_d[ss, :], in_=ys)

        # ---------------- need-weighted interleaved emission ----------------
        def interleave(main, filler):
            """main/filler: lists of (weight, fn). Filler cost is consumed
            proportionally to accumulated main weight, so segments with more
            dependency latency (diagonal blocks, norms) get denser filler."""
            tm = sum(c for c, _ in main) or 1
            tf = sum(c for c, _ in filler)
            rho = tf / tm
            fs = 0.0
            ms = 0.0
            fi = 0
            for c, fn in main:
                fn()
                ms += c
                while fi < len(filler) and fs + filler[fi][0] * 0.5 <= ms * rho:
                    fs += filler[fi][0]
                    filler[fi][1]()
                    fi += 1
            while fi < len(filler):
                filler[fi][1]()
                fi += 1
        CQK = 440  # filler piece cost: 2 N=512 matmuls
        CPROJ = 450  # filler piece cost: 2 N=512 matmuls
        CNORM = 1300
        CNORMA = 200

        def w_sc(v):
            # per-item period is exp-paced: (2w+352)/1.2 minus the PV share
            w = 512 - 128 * v
            return 0.833 * w + 283

        def w_pv(v):
            w = 512 - 128 * v
            return 0.833 * w + 10

        def pipe(j, r):
            """Software-pipelined attention item list for one pair: PV lags
            scores by 2 items so the PE FIFO never blocks on the exp.  The
            norm is NOT included -- it is spliced ~4 items into the next
            pair's stream so its psum slot ("sc" pool) and DVE chain never
            block the next pair's score/exp cadence."""
            nt = 4 * j + 4
            seq = []
            for ti in range(nt):
                v = max(ti - 4 * j, 0)
                seq.append((w_sc(v), lambda j=j, r=r, ti=ti: emit_sc(j, r, ti)))
                if ti >= 3:
                    pv_ti = ti - 3
                    pvv = max(pv_ti - 4 * j, 0)
                    seq.append(
                        (w_pv(pvv), lambda j=j, r=r, t=pv_ti: emit_pv(j, r, t))
                    )
            for pv_ti in (nt - 3, nt - 2, nt - 1):
                pvv = max(pv_ti - 4 * j, 0)
                wt = w_pv(pvv) + (400 if pv_ti == nt - 1 else 0)
                seq.append((wt, lambda j=j, r=r, t=pv_ti: emit_pv(j, r, t)))
            return seq

        def qk_pieces(j, r, which):
            return [
                (CQK, lambda j=j, r=r, w=which, q=q: emit_qk_quarter(j, r, w, q))
                for q in range(4)
            ]

        def v_pieces(j, ii):
            return [
                (CQK, lambda j=j, ii=ii, q=q: emit_v_quarter(j, ii, q))
                for q in range(4)
            ]

        def proj_pieces(j, sb, f):
            return [
                (CPROJ, lambda j=j, sb=sb, f=f, rr=rr: emit_proj_piece(j, sb, f, rr))
                for rr in ((0, 1), (2, 3))
            ]

        # chunk 0: Q first, ordered so compute starts when the first half
        # of wq and xts[0] has landed, while keeping at most two open
        # accumulation groups (mm512 pool is bufs=2)
        for r0, r1 in ((0, 1), (2, 3)):
            for qi in range(4):
                emit_qk_quarter(0, r0, "q", qi)
                emit_qk_quarter(0, r1, "q", qi)

        norm_carry = None  # deferred norm of the previous pair/chunk
        for j in range(SCH):
            # K(j)/V(j) must precede chunk-j diagonal blocks in program
            # order.  Chunk 0: r0's attention is zipped directly into the
            # K/V stream so ACT starts as soon as K(r0)/V(0..3) land.
            # Chunk 1: K(1)/V(1) already ran as chunk-0 filler, so phase 1
            # is just the non-diag pipeline.  Chunks 2-3: zip K/V into r0's
            # non-diagonal pipeline as before.
            if j == 0:
                for q in range(4):
                    emit_qk_quarter(0, 0, "k", q)
                for q in range(4):
                    emit_qk_quarter(0, 1, "k", q)
                emit_sc(0, 0, 0)
                emit_sc(0, 0, 1)
                for q in range(4):
                    emit_v_quarter(0, 0, q)
                for q in range(4):
                    emit_v_quarter(0, 1, q)
                emit_sc(0, 0, 2)
                emit_sc(0, 0, 3)
                emit_pv(0, 0, 0)
                emit_pv(0, 0, 1)
                for q in range(4):
                    emit_v_quarter(0, 2, q)
                for q in range(4):
                    emit_v_quarter(0, 3, q)
                emit_pv(0, 0, 2)
                emit_pv(0, 0, 3)
                for q in range(4):
                    emit_qk_quarter(0, 2, "k", q)
                for q in range(4):
                    emit_qk_quarter(0, 3, "k", q)
                rest0 = []
            else:
                p0 = pipe(j, 0)
                # entries per sc(ti): 1 for ti<3, else 2 (sc + lagged pv)
                ndlen = 4 * j + max(0, 4 * j - 3)
                nd0, rest0 = p0[:ndlen], p0[ndlen:]
                if norm_carry is not None:
                    nd0.insert(min(3, len(nd0)), norm_carry)
                    norm_carry = None
                kv = []
                for rp in ((0, 1), (2, 3)):
                    for qi in range(4):
                        kv.append(qk_pieces(j, rp[0], "k")[qi])
                        kv.append(qk_pieces(j, rp[1], "k")[qi])
                for ip in ((0, 1), (2, 3)):
                    for qi in range(4):
                        kv.append(v_pieces(j, ip[0])[qi])
                        kv.append(v_pieces(j, ip[1])[qi])
                interleave(nd0, kv)
            # rest: r0 diagonal, then r1..r3; each pair's norm is spliced
            # ~4 entries into the NEXT pair's stream
            main = list(rest0)
            for r in range(1, PAIRS):
                pr_items = pipe(j, r)
                if j == 3 and r == 3:
                    # drop the trailing pv(14)/pv(15): re-emitted below
                    # around the half-norms
                    pr_items = pr_items[:-2]
                pr_items.insert(
                    min(4, len(pr_items)),
                    (CNORM, lambda j=j, r=r - 1: emit_norm(j, r)),
                )
                if j == 3 and r == 2:
                    # after norm(3,1) both r0/r1 cat chunks exist: stash
                    # r0+r1 proj partials for the final s-chunk
                    for u in range(8):
                        pr_items.insert(
                            5 + u, (CPROJ, lambda u=u: emit_proj3_a(u))
                        )
                if j == 3 and r == 3:
                    # fold r2 into the stash so only r3 remains at the tail
                    for u in range(8):
                        pr_items.insert(
                            5 + u, (CPROJ, lambda u=u: emit_proj3_c(u))
                        )
                    # pipelined tail: half-norm 0 + its proj3b units run
                    # before pv(14)/pv(15); then half 1 + the rest
                    pr_items.append((CNORM, lambda: emit_norm33_half(0)))
                    for u in range(4):
                        pr_items.append(
                            (0, lambda u=u: emit_proj3_b(u))
                        )
                    for t in (14, 15):
                        pr_items.append(
                            (0, lambda t=t: emit_pv(3, 3, t))
                        )
                    pr_items.append((0, lambda: emit_norm33_half(1)))
                    for u in range(4, 8):
                        pr_items.append(
                            (0, lambda u=u: emit_proj3_b(u))
                        )
                main += pr_items
            if j + 1 < SCH:
                norm_carry = (CNORM, lambda j=j: emit_norm(j, PAIRS - 1))
            filler = []
            if j + 1 < SCH:
                for rp in ((0, 1), (2, 3)):
                    for qi in range(4):
                        filler.append(qk_pieces(j + 1, rp[0], "q")[qi])
                        filler.append(qk_pieces(j + 1, rp[1], "q")[qi])
            if j == 2:
                for sb in range(0, 6):
                    for f in range(2):
                        filler += proj_pieces(sb // 4, sb, f)
            if j == 3:
                for sb in range(6, 12):
                    for f in range(2):
                        filler += proj_pieces(sb // 4, sb, f)
            interleave(main, filler)

    nc.compile()
    return nc


def get_nc():
    if "nc" not in _CACHE:
        _CACHE["nc"] = _build()
    return _CACHE["nc"]


def prep_core_inputs(x, Wq, bq, Wk, bk, Wv, bv, Wp, core):
    """Pack the full-model inputs into one core's input map."""
    b, g = core // 2, core % 2
    heads = list(range(g * 8, g * 8 + 8))

    def pack_w(W):  # [H,E,D] -> local [E, 512] -> [128, 8, 512]
        Wl = np.concatenate([W[h] for h in heads], axis=1)
        return host_round(Wl.reshape(ET, 128, 512).transpose(1, 0, 2))

    wp_l = host_round(
        Wp[g * 512 : (g + 1) * 512].reshape(PAIRS, 128, E).transpose(1, 0, 2)
    )

    def pack_b(bias):
        return np.stack(
            [
                np.concatenate([bias[heads[2 * r]], bias[heads[2 * r + 1]]])
                for r in range(PAIRS)
            ],
            axis=1,
        ).astype(np.float32)

    bv_cat = np.concatenate([bv[h] for h in heads]).astype(np.float32)

    e2 = np.zeros((128, 128), np.float32)
    e2[64, 0:64] = 1.0
    e2[65, 64:128] = 1.0

    p = np.arange(128)[:, None, None]
    v = np.arange(4)[None, :, None]
    c = np.arange(512)[None, None, :]
    mask = (c >= p + 128 * v).astype(np.float32)  # [128, 4, 512]
    mask = host_round(np.repeat(mask[:, :, None, :], 2, axis=2))

    xe = x[b].T.reshape(ET, 128, SCH, 512)  # [et, p, j, s]
    return {
        "xt": host_round(np.ascontiguousarray(xe.transpose(2, 1, 0, 3))),
        "wq": pack_w(Wq),
        "wk": pack_w(Wk),
        "wv": pack_w(Wv),
        "wp": wp_l,
        "bq": pack_b(bq),
        "bk": pack_b(bk),
        "bvb": np.tile(bv_cat[None, :], (128, 1)),
        "mask": mask,
        "eye2": host_round(e2),
    }


def kernel(**inputs):
    from concourse.bass_utils import run_bass_kernel_spmd

    args = {k: np.asarray(v, np.float32) for k, v in inputs.items()}
    nc = get_nc()
    in_maps = [
        prep_core_inputs(
            args["x"], args["Wq"], args["bq"], args["Wk"], args["bk"],
            args["Wv"], args["bv"], args["Wp"], c,
        )
        for c in range(NCORES)
    ]
    res = run_bass_kernel_spmd(nc, in_maps, core_ids=list(range(NCORES)))
    parts = [np.asarray(r["y"], np.float32) for r in res.results]
    out = np.stack([parts[2 * b] + parts[2 * b + 1] for b in range(B)])
    return (out + args["bp"][None, None, :]).astype(np.float32)


# revision 45
# speedup vs baseline: 1.0122x; 1.0122x over previous
"""Multi-head causal attention (B=4, S=2048, E=1024, H=16, D=64) on 8 trn2 cores.

Sharding: core c handles batch b = c//2 and head-group g = c%2 (8 heads each).
Each core computes its partial output projection over its 512 local concat
columns; the host sums the two partials per batch and adds bp.

Layout strategy (per core):
  - x is pre-transposed on host: xT [1024(+pad), S] as [128, 8, S] e-tiles.
  - Q^T, K^T computed as [d, s] (d on partitions, 2 heads per 128-partition
    pair tile) so scores come out transposed: scoresT [t, s].  The two heads
    of a pair sit at partitions 0-63 / 64-127, so the score matmuls of a pair
    run concurrently on different PE row groups.
  - V kept natural [t, d] with a ones column per head (66-col stride), so the
    PV matmul also produces the softmax denominator as row 64/65 of its
    output.  V bias is added on DVE from a host-broadcast tile.
  - Softmax: exp on ACT (no max subtraction -- scores are O(1) by
    construction), causal masking via ONE broadcast multiplicative 0/1 mask
    on DVE per diag item, both denominators merged to rows 64/65 with a
    single DVE add, broadcast across partitions via a K=2 matmul,
    reciprocal on DVE, normalize into concat^T, output projection from
    concat^T, y output in bf16 (host does f32 sum).
  - Attention is software-pipelined: the PV matmuls of item ti are emitted
    after the score matmuls of item ti+2, so the PE FIFO never head-blocks
    on the ACT exp of the current item (exp latency ~1.15us/item is the
    pacer in late chunks; the 2-item lag lets scores+fillers run under it).
  - Scheduling: attention items are interleaved with "filler" pieces (2
    matmuls each) from the QKV projections of later chunks and the output
    projection of earlier chunks, spread by estimated cost so the tensor
    engine never starves while ACT/DVE work through the softmax chain.
  - Head: first DMAs are split into 256KB et-pair slices across both HWDGE
    queues so the first Q matmuls start ~9us instead of ~14us; the 2MB wp
    (not needed until chunk 2) is deferred to the queue tails; y outputs go
    out on the HWDGE queues (idle after input load) instead of SWDGE.
"""

import numpy as np

B, S, E, H, D = 4, 2048, 1024, 16, 64
NCORES = 8
PAIRS = 4  # head pairs per core (8 heads)
ET = 8  # e-tiles of 128 for the contraction over E
SCH = 4  # s-chunks of 512
VW = 66  # V columns per head: 64 d + 1 ones + 1 pad
SCALE = float(D) ** -0.5

MM_DTYPE = "bfloat16"

_CACHE = {}


def host_round(a):
    import ml_dtypes

    return np.ascontiguousarray(a, np.float32).astype(ml_dtypes.bfloat16)


def _build():
    import concourse.tile as tile
    from concourse import bacc, mybir
    from contextlib import ExitStack

    f32 = mybir.dt.float32
    bf16 = mybir.dt.bfloat16
    mdt = getattr(mybir.dt, MM_DTYPE)
    AF = mybir.ActivationFunctionType

    nc = bacc.Bacc("TRN2", target_bir_lowering=False, debug=False, num_devices=NCORES)

    xt_d = nc.dram_tensor("xt", [SCH, 128, ET, 512], mdt, kind="ExternalInput").ap()
    wq_d = nc.dram_tensor("wq", [128, ET, 512], mdt, kind="ExternalInput").ap()
    wk_d = nc.dram_tensor("wk", [128, ET, 512], mdt, kind="ExternalInput").ap()
    wv_d = nc.dram_tensor("wv", [128, ET, 512], mdt, kind="ExternalInput").ap()
    wp_d = nc.dram_tensor("wp", [128, PAIRS, E], mdt, kind="ExternalInput").ap()
    bq_d = nc.dram_tensor("bq", [128, PAIRS], f32, kind="ExternalInput").ap()
    bk_d = nc.dram_tensor("bk", [128, PAIRS], f32, kind="ExternalInput").ap()
    bvb_d = nc.dram_tensor("bvb", [128, 512], f32, kind="ExternalInput").ap()
    mask_d = nc.dram_tensor("mask", [128, 4, 2, 512], mdt, kind="ExternalInput").ap()
    eye2_d = nc.dram_tensor("eye2", [128, 128], mdt, kind="ExternalInput").ap()
    y_d = nc.dram_tensor("y", [S, E], bf16, kind="ExternalOutput").ap()

    with tile.TileContext(nc) as tc, ExitStack() as ctx:
        pers = ctx.enter_context(tc.tile_pool(name="pers", bufs=1))
        work = ctx.enter_context(tc.tile_pool(name="work", bufs=1))
        psp = ctx.enter_context(tc.tile_pool(name="psp", bufs=1, space="PSUM"))

        qt = pers.tile([128, PAIRS, S], mdt)  # Q^T pair tiles
        kt = pers.tile([128, PAIRS, S], mdt)  # K^T pair tiles
        va = pers.tile([128, 16, 8 * VW], mdt)  # V (+ones col) per t-block
        cat = pers.tile([128, PAIRS, S], mdt)  # concat^T
        bq_sb = pers.tile([128, PAIRS], f32)
        bk_sb = pers.tile([128, PAIRS], f32)
        bvb_sb = pers.tile([128, 512], f32)
        eye2 = pers.tile([128, 128], mdt)  # bc lhsT: rows 64/65 block-select
        wq_sb = pers.tile([128, ET, 512], mdt)
        wk_sb = pers.tile([128, ET, 512], mdt)
        wv_sb = pers.tile([128, ET, 512], mdt)
        wp_sb = pers.tile([128, PAIRS, E], mdt)
        mask_sb = pers.tile([128, 4, 2, 512], mdt)  # mask duplicated per head
        y01 = pers.tile([128, 8, 512], f32)  # chunk-3 proj r0+r1 partials
        xts = [
            work.tile([128, ET, 512], mdt, tag=f"xt{j % 2}", name=f"xt{j}")
            for j in range(SCH)
        ]

        # ---- head DMAs: only sync/scalar (HWDGE) + gpsimd (SWDGE) can issue.
        # First-needed data goes first in 256KB et-pair slices alternating
        # across the two HWDGE queues so Q(0) compute can start as soon as
        # the first pair lands.  wp (2MB, needed only from chunk 2) is
        # deferred to the queue tails so it doesn't steal HBM bandwidth from
        # the critical path.  gpsimd (SWDGE) only gets the small tensors.
        nc.sync.dma_start(out=wq_sb[:, 0:2, :], in_=wq_d[:, 0:2, :])
        nc.scalar.dma_start(out=xts[0][:, 0:2, :], in_=xt_d[0][:, 0:2, :])
        nc.sync.dma_start(out=wq_sb[:, 2:4, :], in_=wq_d[:, 2:4, :])
        nc.scalar.dma_start(out=xts[0][:, 2:4, :], in_=xt_d[0][:, 2:4, :])
        nc.sync.dma_start(out=xts[0][:, 4:8, :], in_=xt_d[0][:, 4:8, :])
        nc.scalar.dma_start(out=wq_sb[:, 4:8, :], in_=wq_d[:, 4:8, :])
        nc.sync.dma_start(out=wk_sb[:, 0:4, :], in_=wk_d[:, 0:4, :])
        nc.scalar.dma_start(out=wk_sb[:, 4:8, :], in_=wk_d[:, 4:8, :])
        nc.gpsimd.dma_start(out=eye2, in_=eye2_d)
        nc.gpsimd.dma_start(out=bq_sb, in_=bq_d)
        nc.gpsimd.dma_start(out=bk_sb, in_=bk_d)
        nc.gpsimd.dma_start(out=bvb_sb, in_=bvb_d)
        nc.gpsimd.dma_start(out=mask_sb, in_=mask_d)
        nc.sync.dma_start(out=wv_sb[:, 0:4, :], in_=wv_d[:, 0:4, :])
        nc.scalar.dma_start(out=wv_sb[:, 4:8, :], in_=wv_d[:, 4:8, :])
        nc.sync.dma_start(out=xts[1][:, 0:4, :], in_=xt_d[1][:, 0:4, :])
        nc.scalar.dma_start(out=xts[1][:, 4:8, :], in_=xt_d[1][:, 4:8, :])
        nc.sync.dma_start(out=xts[2][:, 0:4, :], in_=xt_d[2][:, 0:4, :])
        nc.scalar.dma_start(out=xts[2][:, 4:8, :], in_=xt_d[2][:, 4:8, :])
        nc.sync.dma_start(out=xts[3][:, 0:4, :], in_=xt_d[3][:, 0:4, :])
        nc.scalar.dma_start(out=xts[3][:, 4:8, :], in_=xt_d[3][:, 4:8, :])
        nc.sync.dma_start(out=wp_sb[:, 0:2, :], in_=wp_d[:, 0:2, :])
        nc.scalar.dma_start(out=wp_sb[:, 2:4, :], in_=wp_d[:, 2:4, :])

        # ones + pad columns of va are static: even heads carry ones at col
        # 64 (denominator -> PSUM row 64), odd heads at col 65 (-> row 65),
        # so the two denominators land on rows 64/65 of the two PV psums and
        # merge with a single DVE add
        va_hc = va.rearrange("p i (h c) -> p i h c", c=VW)
        va_pp = va.rearrange("p i (g w) -> p (i g) w", w=2 * VW)
        nc.vector.memset(va_pp[:, :, 64:65], 1.0)
        nc.vector.memset(va_pp[:, :, 65:66], 0.0)
        nc.vector.memset(va_pp[:, :, VW + 64 : VW + 65], 0.0)
        nc.vector.memset(va_pp[:, :, VW + 65 : VW + 66], 1.0)

        # ---------------- work-item emitters ----------------
        qk_state = {}

        def emit_qk_quarter(j, r, which, qi):
            """Quarter of a Q/K projection unit: 2 e-tile matmuls; the last
            quarter finishes the accumulation and adds the bias on DVE."""
            w_sb, dst, b_sb = (
                (wq_sb, qt, bq_sb) if which == "q" else (wk_sb, kt, bk_sb)
            )
            key = (j, r, which)
            if qi == 0:
                qk_state[key] = psp.tile(
                    [128, 512], f32, tag="mm512", bufs=2, name="qkps"
                )
            ps = qk_state[key]
            for et in range(2 * qi, 2 * qi + 2):
                nc.tensor.matmul(
                    ps,
                    lhsT=w_sb[:, et, r * 128 : (r + 1) * 128],
                    rhs=xts[j][:, et, :],
                    start=(et == 0),
                    stop=(et == ET - 1),
                )
            if qi == 3:
                del qk_state[key]
                sjl = slice(j * 512, (j + 1) * 512)
                nc.vector.tensor_scalar_add(
                    dst[:, r, sjl], ps, b_sb[:, r : r + 1]
                )

        def emit_v_quarter(j, ii, qi):
            i = 4 * j + ii
            si = slice(ii * 128, (ii + 1) * 128)
            key = ("v", j, ii)
            if qi == 0:
                qk_state[key] = psp.tile(
                    [128, 512], f32, tag="mm512", bufs=2, name="vps"
                )
            ps = qk_state[key]
            for et in range(2 * qi, 2 * qi + 2):
                nc.tensor.matmul(
                    ps,
                    lhsT=xts[j][:, et, si],
                    rhs=wv_sb[:, et, :],
                    start=(et == 0),
                    stop=(et == ET - 1),
                )
            if qi == 3:
                del qk_state[key]
                va_i = va_hc[:, i]
                nc.vector.tensor_tensor(
                    va_i[:, :, 0:64],
                    ps.rearrange("p (h d) -> p h d", d=64),
                    bvb_sb.rearrange("p (h d) -> p h d", d=64),
                    op=mybir.AluOpType.add,
                )

        attn_state = {}
        attn_pr = {}

        def emit_sc(j, r, ti):
            """Score pair for item ti + exp on ACT (+ causal mask on DVE for
            diagonal items).  PV is emitted separately, 2 items later."""
            if ti == 0:
                attn_state[(j, r)] = [
                    psp.tile([VW, 512], f32, tag=f"o{hh}", bufs=1,
                             name=f"outp{hh}")
                    for hh in range(2)
                ]
            tis = slice(ti * 128, (ti + 1) * 128)
            v = max(ti - 4 * j, 0)
            w = 512 - 128 * v
            sjv = slice(j * 512 + 128 * v, (j + 1) * 512)
            scp = psp.tile([128, 2, 512], f32, tag="sc", bufs=2)
            for hh in range(2):
                po = hh * 64
                nc.tensor.matmul(
                    scp[:, hh, 128 * v :],
                    lhsT=kt[po : po + 64, r, tis],
                    rhs=qt[po : po + 64, r, sjv],
                    start=True,
                    stop=True,
                )
            pr = work.tile([128, 2, 512], mdt, tag="pr", bufs=6)
            nc.scalar.activation(
                pr[:, :, 128 * v :], scp[:, :, 128 * v :], AF.Exp, scale=SCALE
            )
            if v or ti == 4 * j:
                nc.vector.tensor_tensor(
                    pr[:, :, 128 * v :], pr[:, :, 128 * v :],
                    mask_sb[:, v, :, 128 * v :],
                    op=mybir.AluOpType.mult,
                )
            attn_pr[(j, r, ti)] = pr

        def emit_pv(j, r, ti):
            nt = 4 * j + 4
            outps = attn_state[(j, r)]
            pr = attn_pr.pop((j, r, ti))
            v = max(ti - 4 * j, 0)
            for hh in range(2):
                h = 2 * r + hh
                nc.tensor.matmul(
                    outps[hh][:, 128 * v :],
                    lhsT=va[:, ti, h * VW : (h + 1) * VW],
                    rhs=pr[:, hh, 128 * v :],
                    start=(ti == 0),
                    stop=(ti == nt - 1),
                )

        def emit_norm33_half(h):
            """Half-norm for the last pair (3,3): columns [256h, 256h+256).
            Half 0 is final after pv(13), so it (and its proj3b units) is
            emitted BEFORE pv(14)/pv(15) -- Tile serializes the psum-bank
            reads against the remaining PV writes, pipelining the tail."""
            outps = attn_state[(3, 3)]
            cs = slice(256 * h, 256 * h + 256)
            sjl = slice(3 * 512 + 256 * h, 3 * 512 + 256 * h + 256)
            osbd = work.tile([128, 512], mdt, tag="osbd", bufs=2)
            nc.vector.tensor_copy(osbd[64:66, cs], outps[1][64:66, cs])
            nc.vector.tensor_copy(osbd[64:65, cs], outps[0][64:65, cs])
            rdp = psp.tile([128, 512], f32, tag="sc", bufs=2, name="bcst")
            nc.tensor.matmul(
                rdp[:, cs],
                lhsT=eye2[64:66, :],
                rhs=osbd[64:66, cs],
                start=True,
                stop=True,
            )
            rd = work.tile([128, 512], f32, tag="rd", bufs=2)
            nc.vector.reciprocal_approx_fast(rd[:, cs], rdp[:, cs])
            nc.vector.tensor_mul(
                cat[0:64, 3, sjl], outps[0][0:64, cs], rd[0:64, cs]
            )
            nc.vector.tensor_mul(
                cat[64:128, 3, sjl], outps[1][0:64, cs], rd[64:128, cs]
            )
            if h == 1:
                attn_state.pop((3, 3))

        norm_state = {}

        def emit_norm_a(j, r):
            """Denominator copies (DVE only) -- emitted early so the bcst
            matmul in emit_norm_b never exposes the DVE latency on PE."""
            outps = attn_state[(j, r)]
            # both denominators to adjacent partitions 64/65, then ONE K=2
            # matmul broadcasts h0's to rows 0-63 and h1's to rows 64-127
            osbd = work.tile([128, 512], mdt, tag="osbd", bufs=2)
            nc.vector.tensor_copy(osbd[64:66, :], outps[1][64:66, :])
            nc.vector.tensor_copy(osbd[64:65, :], outps[0][64:65, :])
            norm_state[(j, r)] = osbd

        def emit_norm_b(j, r):
            outps = attn_state.pop((j, r))
            osbd = norm_state.pop((j, r))
            sjl = slice(j * 512, (j + 1) * 512)
            rdp = psp.tile([128, 512], f32, tag="sc", bufs=2, name="bcst")
            nc.tensor.matmul(
                rdp,
                lhsT=eye2[64:66, :],
                rhs=osbd[64:66, :],
                start=True,
                stop=True,
            )
            # normalize directly from the PV psum (mixed PSUM+SB inputs are
            # exempt from the equal-base-partition rule)
            rd = work.tile([128, 512], f32, tag="rd", bufs=2)
            nc.vector.reciprocal_approx_fast(rd, rdp)
            nc.vector.tensor_mul(
                cat[0:64, r, sjl], outps[0][0:64, :], rd[0:64, :]
            )
            nc.vector.tensor_mul(
                cat[64:128, r, sjl], outps[1][0:64, :], rd[64:128, :]
            )

        def emit_norm(j, r):
            emit_norm_a(j, r)
            emit_norm_b(j, r)

        def emit_proj_piece(j, sb, f, rr, on_act=False):
            """Output-projection piece: contraction pairs rr=(0,1) or (2,3).
            Tail-woven pieces evacuate psum on the (idle) scalar engine so
            the tail's DVE chain is untouched."""
            ss = slice(sb * 128, (sb + 1) * 128)
            sf = slice(f * 512, (f + 1) * 512)
            key = ("p", j, sb, f)
            if rr[0] == 0:
                qk_state[key] = psp.tile(
                    [128, 512], f32, tag="mm512", bufs=2, name="yproj"
                )
            yp = qk_state[key]
            for r in rr:
                nc.tensor.matmul(
                    yp,
                    lhsT=cat[:, r, ss],
                    rhs=wp_sb[:, r, sf],
                    start=(r == 0),
                    stop=(r == PAIRS - 1),
                )
            if rr[-1] == PAIRS - 1:
                del qk_state[key]
                ykey = ("ys", sb)
                if f == 0:
                    qk_state[ykey] = work.tile(
                        [128, 2, 512], mdt, tag="ys", bufs=2, name="yspair"
                    )
                ys = qk_state[ykey]
                if on_act:
                    nc.scalar.copy(ys[:, f, :], yp)
                else:
                    nc.vector.tensor_copy(ys[:, f, :], yp)
                if f == 1:
                    del qk_state[ykey]
                    nc.sync.dma_start(out=y_d[ss, :], in_=ys)

        def emit_proj3_a(u):
            """Chunk-3 proj unit u (sb=12+u//2, f=u%2): r0+r1 into stash."""
            sb, f = 12 + u // 2, u % 2
            ss = slice(sb * 128, (sb + 1) * 128)
            sf = slice(f * 512, (f + 1) * 512)
            yp = psp.tile([128, 512], f32, tag="mm512", bufs=2, name="yp3a")
            for r in (0, 1):
                nc.tensor.matmul(
                    yp,
                    lhsT=cat[:, r, ss],
                    rhs=wp_sb[:, r, sf],
                    start=(r == 0),
                    stop=(r == 1),
                )
            nc.vector.tensor_copy(y01[:, u, :], yp)

        def emit_proj3_c(u):
            """Chunk-3 proj unit u: fold r2 into the stash after norm(3,2)."""
            sb, f = 12 + u // 2, u % 2
            ss = slice(sb * 128, (sb + 1) * 128)
            sf = slice(f * 512, (f + 1) * 512)
            yp = psp.tile([128, 512], f32, tag="mm512", bufs=2, name="yp3c")
            nc.tensor.matmul(
                yp, lhsT=cat[:, 2, ss], rhs=wp_sb[:, 2, sf], start=True,
                stop=True,
            )
            nc.vector.tensor_tensor(
                y01[:, u, :], y01[:, u, :], yp, op=mybir.AluOpType.add
            )

        def emit_proj3_b(u):
            """Chunk-3 proj unit u: r3 only, add stash, emit output."""
            sb, f = 12 + u // 2, u % 2
            ss = slice(sb * 128, (sb + 1) * 128)
            sf = slice(f * 512, (f + 1) * 512)
            yp = psp.tile([128, 512], f32, tag="mm512", bufs=2, name="yp3b")
            nc.tensor.matmul(
                yp, lhsT=cat[:, 3, ss], rhs=wp_sb[:, 3, sf], start=True,
                stop=True,
            )
            ykey = ("ys3", sb)
            if f == 0:
                qk_state[ykey] = work.tile(
                    [128, 2, 512], mdt, tag="ys", bufs=2, name="yspair3"
                )
            ys = qk_state[ykey]
            nc.vector.tensor_tensor(
                ys[:, f, :], yp, y01[:, u, :], op=mybir.AluOpType.add
            )
            if f == 1:
                del qk_state[ykey]
                nc.sync.dma_start(out=y_d[ss, :], in_=ys)

        # ---------------- need-weighted interleaved emission ----------------
        def interleave(main, filler):
            """main/filler: lists of (weight, fn). Filler cost is consumed
            proportionally to accumulated main weight, so segments with more
            dependency latency (diagonal blocks, norms) get denser filler."""
            tm = sum(c for c, _ in main) or 1
            tf = sum(c for c, _ in filler)
            rho = tf / tm
            fs = 0.0
            ms = 0.0
            fi = 0
            for c, fn in main:
                fn()
                ms += c
                while fi < len(filler) and fs + filler[fi][0] * 0.5 <= ms * rho:
                    fs += filler[fi][0]
                    filler[fi][1]()
                    fi += 1
            while fi < len(filler):
                filler[fi][1]()
                fi += 1
        CQK = 440  # filler piece cost: 2 N=512 matmuls
        CPROJ = 450  # filler piece cost: 2 N=512 matmuls
        CNORM = 1300
        CNORMA = 200

        def w_sc(v):
            # per-item period is exp-paced: (2w+352)/1.2 minus the PV share
            w = 512 - 128 * v
            return 0.833 * w + 283

        def w_pv(v):
            w = 512 - 128 * v
            return 0.833 * w + 10

        def pipe(j, r):
            """Software-pipelined attention item list for one pair: PV lags
            scores by 2 items so the PE FIFO never blocks on the exp.  The
            norm is NOT included -- it is spliced ~4 items into the next
            pair's stream so its psum slot ("sc" pool) and DVE chain never
            block the next pair's score/exp cadence."""
            nt = 4 * j + 4
            seq = []
            for ti in range(nt):
                v = max(ti - 4 * j, 0)
                seq.append((w_sc(v), lambda j=j, r=r, ti=ti: emit_sc(j, r, ti)))
                if ti >= 2:
                    pv_ti = ti - 2
                    pvv = max(pv_ti - 4 * j, 0)
                    seq.append(
                        (w_pv(pvv), lambda j=j, r=r, t=pv_ti: emit_pv(j, r, t))
                    )
            for pv_ti in (nt - 2, nt - 1):
                pvv = max(pv_ti - 4 * j, 0)
                wt = w_pv(pvv) + (400 if pv_ti == nt - 1 else 0)
                seq.append((wt, lambda j=j, r=r, t=pv_ti: emit_pv(j, r, t)))
            return seq

        def qk_pieces(j, r, which):
            return [
                (CQK, lambda j=j, r=r, w=which, q=q: emit_qk_quarter(j, r, w, q))
                for q in range(4)
            ]

        def v_pieces(j, ii):
            return [
                (CQK, lambda j=j, ii=ii, q=q: emit_v_quarter(j, ii, q))
                for q in range(4)
            ]

        def proj_pieces(j, sb, f):
            return [
                (CPROJ, lambda j=j, sb=sb, f=f, rr=rr: emit_proj_piece(j, sb, f, rr))
                for rr in ((0, 1), (2, 3))
            ]

        # chunk 0: Q first, ordered so compute starts when the first half
        # of wq and xts[0] has landed, while keeping at most two open
        # accumulation groups (mm512 pool is bufs=2)
        for r0, r1 in ((0, 1), (2, 3)):
            for qi in range(4):
                emit_qk_quarter(0, r0, "q", qi)
                emit_qk_quarter(0, r1, "q", qi)

        norm_carry = None  # deferred norm of the previous pair/chunk
        for j in range(SCH):
            # K(j)/V(j) must precede chunk-j diagonal blocks in program
            # order.  Chunk 0: r0's attention is zipped directly into the
            # K/V stream so ACT starts as soon as K(r0)/V(0..3) land.
            # Chunk 1: K(1)/V(1) already ran as chunk-0 filler, so phase 1
            # is just the non-diag pipeline.  Chunks 2-3: zip K/V into r0's
            # non-diagonal pipeline as before.
            if j == 0:
                for q in range(4):
                    emit_qk_quarter(0, 0, "k", q)
                for q in range(4):
                    emit_qk_quarter(0, 1, "k", q)
                emit_sc(0, 0, 0)
                emit_sc(0, 0, 1)
                for q in range(4):
                    emit_v_quarter(0, 0, q)
                for q in range(4):
                    emit_v_quarter(0, 1, q)
                emit_sc(0, 0, 2)
                emit_sc(0, 0, 3)
                emit_pv(0, 0, 0)
                emit_pv(0, 0, 1)
                for q in range(4):
                    emit_v_quarter(0, 2, q)
                for q in range(4):
                    emit_v_quarter(0, 3, q)
                emit_pv(0, 0, 2)
                emit_pv(0, 0, 3)
                for q in range(4):
                    emit_qk_quarter(0, 2, "k", q)
                for q in range(4):
                    emit_qk_quarter(0, 3, "k", q)
                rest0 = []
            else:
                p0 = pipe(j, 0)
                # entries per sc(ti): 1 for ti<2, else 2 (sc + lagged pv)
                ndlen = 4 * j + max(0, 4 * j - 2)
                nd0, rest0 = p0[:ndlen], p0[ndlen:]
                if norm_carry is not None:
                    nd0.insert(min(3, len(nd0)), norm_carry)
                    norm_carry = None
                kv = []
                for rp in ((0, 1), (2, 3)):
                    for qi in range(4):
                        kv.append(qk_pieces(j, rp[0], "k")[qi])
                        kv.append(qk_pieces(j, rp[1], "k")[qi])
                for ip in ((0, 1), (2, 3)):
                    for qi in range(4):
                        kv.append(v_pieces(j, ip[0])[qi])
                        kv.append(v_pieces(j, ip[1])[qi])
                interleave(nd0, kv)
            # rest: r0 diagonal, then r1..r3; each pair's norm is spliced
            # ~4 entries into the NEXT pair's stream
            main = list(rest0)
            for r in range(1, PAIRS):
                pr_items = pipe(j, r)
                if j == 3 and r == 3:
                    # drop the trailing pv(14)/pv(15): re-emitted below
                    # around the half-norms
                    pr_items = pr_items[:-2]
                pr_items.insert(
                    min(4, len(pr_items)),
                    (CNORM, lambda j=j, r=r - 1: emit_norm(j, r)),
                )
                if j == 3 and r == 2:
                    # after norm(3,1) both r0/r1 cat chunks exist: stash
                    # r0+r1 proj partials for the final s-chunk
                    for u in range(8):
                        pr_items.insert(
                            5 + u, (CPROJ, lambda u=u: emit_proj3_a(u))
                        )
                if j == 3 and r == 3:
                    # fold r2 into the stash so only r3 remains at the tail
                    for u in range(8):
                        pr_items.insert(
                            5 + u, (CPROJ, lambda u=u: emit_proj3_c(u))
                        )
                    # pipelined tail: half-norm 0 + its proj3b units run
                    # before pv(14)/pv(15); then half 1 + the rest
                    pr_items.append((CNORM, lambda: emit_norm33_half(0)))
                    for u in range(4):
                        pr_items.append(
                            (0, lambda u=u: emit_proj3_b(u))
                        )
                    for t in (14, 15):
                        pr_items.append(
                            (0, lambda t=t: emit_pv(3, 3, t))
                        )
                    pr_items.append((0, lambda: emit_norm33_half(1)))
                    for u in range(4, 8):
                        pr_items.append(
                            (0, lambda u=u: emit_proj3_b(u))
                        )
                main += pr_items
            if j + 1 < SCH:
                norm_carry = (CNORM, lambda j=j: emit_norm(j, PAIRS - 1))
            filler = []
            if j + 1 < SCH:
                for rp in ((0, 1), (2, 3)):
                    for qi in range(4):
                        filler.append(qk_pieces(j + 1, rp[0], "q")[qi])
                        filler.append(qk_pieces(j + 1, rp[1], "q")[qi])
            if j == 2:
                for sb in range(0, 6):
                    for f in range(2):
                        filler += proj_pieces(sb // 4, sb, f)
            if j == 3:
                for sb in range(6, 12):
                    for f in range(2):
                        filler += proj_pieces(sb // 4, sb, f)
            interleave(main, filler)

    nc.compile()
    return nc


def get_nc():
    if "nc" not in _CACHE:
        _CACHE["nc"] = _build()
    return _CACHE["nc"]


def prep_core_inputs(x, Wq, bq, Wk, bk, Wv, bv, Wp, core):
    """Pack the full-model inputs into one core's input map."""
    b, g = core // 2, core % 2
    heads = list(range(g * 8, g * 8 + 8))

    def pack_w(W):  # [H,E,D] -> local [E, 512] -> [128, 8, 512]
        Wl = np.concatenate([W[h] for h in heads], axis=1)
        return host_round(Wl.reshape(ET, 128, 512).transpose(1, 0, 2))

    wp_l = host_round(
        Wp[g * 512 : (g + 1) * 512].reshape(PAIRS, 128, E).transpose(1, 0, 2)
    )

    def pack_b(bias):
        return np.stack(
            [
                np.concatenate([bias[heads[2 * r]], bias[heads[2 * r + 1]]])
                for r in range(PAIRS)
            ],
            axis=1,
        ).astype(np.float32)

    bv_cat = np.concatenate([bv[h] for h in heads]).astype(np.float32)

    e2 = np.zeros((128, 128), np.float32)
    e2[64, 0:64] = 1.0
    e2[65, 64:128] = 1.0

    p = np.arange(128)[:, None, None]
    v = np.arange(4)[None, :, None]
    c = np.arange(512)[None, None, :]
    mask = (c >= p + 128 * v).astype(np.float32)  # [128, 4, 512]
    mask = host_round(np.repeat(mask[:, :, None, :], 2, axis=2))

    xe = x[b].T.reshape(ET, 128, SCH, 512)  # [et, p, j, s]
    return {
        "xt": host_round(np.ascontiguousarray(xe.transpose(2, 1, 0, 3))),
        "wq": pack_w(Wq),
        "wk": pack_w(Wk),
        "wv": pack_w(Wv),
        "wp": wp_l,
        "bq": pack_b(bq),
        "bk": pack_b(bk),
        "bvb": np.tile(bv_cat[None, :], (128, 1)),
        "mask": mask,
        "eye2": host_round(e2),
    }


def kernel(**inputs):
    from concourse.bass_utils import run_bass_kernel_spmd

    args = {k: np.asarray(v, np.float32) for k, v in inputs.items()}
    nc = get_nc()
    in_maps = [
        prep_core_inputs(
            args["x"], args["Wq"], args["bq"], args["Wk"], args["bk"],
            args["Wv"], args["bv"], args["Wp"], c,
        )
        for c in range(NCORES)
    ]
    res = run_bass_kernel_spmd(nc, in_maps, core_ids=list(range(NCORES)))
    parts = [np.asarray(r["y"], np.float32) for r in res.results]
    out = np.stack([parts[2 * b] + parts[2 * b + 1] for b in range(B)])
    return (out + args["bp"][None, None, :]).astype(np.float32)


# revision 46
# speedup vs baseline: 1.0216x; 1.0092x over previous
"""Multi-head causal attention (B=4, S=2048, E=1024, H=16, D=64) on 8 trn2 cores.

Sharding: core c handles batch b = c//2 and head-group g = c%2 (8 heads each).
Each core computes its partial output projection over its 512 local concat
columns; the host sums the two partials per batch and adds bp.

Layout strategy (per core):
  - x is pre-transposed on host: xT [1024(+pad), S] as [128, 8, S] e-tiles.
  - Q^T, K^T computed as [d, s] (d on partitions, 2 heads per 128-partition
    pair tile) so scores come out transposed: scoresT [t, s].  The two heads
    of a pair sit at partitions 0-63 / 64-127, so the score matmuls of a pair
    run concurrently on different PE row groups.
  - V kept natural [t, d] with a ones column per head (66-col stride), so the
    PV matmul also produces the softmax denominator as row 64/65 of its
    output.  V bias is added on DVE from a host-broadcast tile.
  - Softmax: exp on ACT (no max subtraction -- scores are O(1) by
    construction), causal masking via ONE broadcast multiplicative 0/1 mask
    on DVE per diag item, both denominators merged to rows 64/65 with a
    single DVE add, broadcast across partitions via a K=2 matmul,
    reciprocal on DVE, normalize into concat^T, output projection from
    concat^T, y output in bf16 (host does f32 sum).
  - Attention is software-pipelined: the PV matmuls of item ti are emitted
    after the score matmuls of item ti+2, so the PE FIFO never head-blocks
    on the ACT exp of the current item (exp latency ~1.15us/item is the
    pacer in late chunks; the 2-item lag lets scores+fillers run under it).
  - Scheduling: attention items are interleaved with "filler" pieces (2
    matmuls each) from the QKV projections of later chunks and the output
    projection of earlier chunks, spread by estimated cost so the tensor
    engine never starves while ACT/DVE work through the softmax chain.
  - Head: first DMAs are split into 256KB et-pair slices across both HWDGE
    queues so the first Q matmuls start ~9us instead of ~14us; the 2MB wp
    (not needed until chunk 2) is deferred to the queue tails; y outputs go
    out on the HWDGE queues (idle after input load) instead of SWDGE.
"""

import numpy as np

B, S, E, H, D = 4, 2048, 1024, 16, 64
NCORES = 8
PAIRS = 4  # head pairs per core (8 heads)
ET = 8  # e-tiles of 128 for the contraction over E
SCH = 4  # s-chunks of 512
VW = 66  # V columns per head: 64 d + 1 ones + 1 pad
SCALE = float(D) ** -0.5

MM_DTYPE = "bfloat16"

_CACHE = {}


def host_round(a):
    import ml_dtypes

    return np.ascontiguousarray(a, np.float32).astype(ml_dtypes.bfloat16)


def _build():
    import concourse.tile as tile
    from concourse import bacc, mybir
    from contextlib import ExitStack

    f32 = mybir.dt.float32
    bf16 = mybir.dt.bfloat16
    mdt = getattr(mybir.dt, MM_DTYPE)
    AF = mybir.ActivationFunctionType

    nc = bacc.Bacc("TRN2", target_bir_lowering=False, debug=False, num_devices=NCORES)

    xt_d = nc.dram_tensor("xt", [SCH, 128, ET, 512], mdt, kind="ExternalInput").ap()
    wq_d = nc.dram_tensor("wq", [128, ET, 512], mdt, kind="ExternalInput").ap()
    wk_d = nc.dram_tensor("wk", [128, ET, 512], mdt, kind="ExternalInput").ap()
    wv_d = nc.dram_tensor("wv", [128, ET, 512], mdt, kind="ExternalInput").ap()
    wp_d = nc.dram_tensor("wp", [128, PAIRS, E], mdt, kind="ExternalInput").ap()
    bq_d = nc.dram_tensor("bq", [128, PAIRS], f32, kind="ExternalInput").ap()
    bk_d = nc.dram_tensor("bk", [128, PAIRS], f32, kind="ExternalInput").ap()
    bvb_d = nc.dram_tensor("bvb", [128, 512], f32, kind="ExternalInput").ap()
    mask_d = nc.dram_tensor("mask", [128, 4, 2, 512], mdt, kind="ExternalInput").ap()
    eye2_d = nc.dram_tensor("eye2", [128, 128], mdt, kind="ExternalInput").ap()
    y_d = nc.dram_tensor("y", [S, E], bf16, kind="ExternalOutput").ap()

    with tile.TileContext(nc) as tc, ExitStack() as ctx:
        pers = ctx.enter_context(tc.tile_pool(name="pers", bufs=1))
        work = ctx.enter_context(tc.tile_pool(name="work", bufs=1))
        psp = ctx.enter_context(tc.tile_pool(name="psp", bufs=1, space="PSUM"))

        qt = pers.tile([128, PAIRS, S], mdt)  # Q^T pair tiles
        kt = pers.tile([128, PAIRS, S], mdt)  # K^T pair tiles
        va = pers.tile([128, 16, 8 * VW], mdt)  # V (+ones col) per t-block
        cat = pers.tile([128, PAIRS, S], mdt)  # concat^T
        bq_sb = pers.tile([128, PAIRS], f32)
        bk_sb = pers.tile([128, PAIRS], f32)
        bvb_sb = pers.tile([128, 512], f32)
        eye2 = pers.tile([128, 128], mdt)  # bc lhsT: rows 64/65 block-select
        wq_sb = pers.tile([128, ET, 512], mdt)
        wk_sb = pers.tile([128, ET, 512], mdt)
        wv_sb = pers.tile([128, ET, 512], mdt)
        wp_sb = pers.tile([128, PAIRS, E], mdt)
        mask_sb = pers.tile([128, 4, 2, 512], mdt)  # mask duplicated per head
        y01 = pers.tile([128, 8, 512], f32)  # chunk-3 proj r0+r1 partials
        xts = [
            work.tile([128, ET, 512], mdt, tag=f"xt{j % 2}", name=f"xt{j}")
            for j in range(SCH)
        ]

        # ---- head DMAs: only sync/scalar (HWDGE) + gpsimd (SWDGE) can issue.
        # First-needed data goes first in 256KB et-pair slices alternating
        # across the two HWDGE queues so Q(0) compute can start as soon as
        # the first pair lands.  wp (2MB, needed only from chunk 2) is
        # deferred to the queue tails so it doesn't steal HBM bandwidth from
        # the critical path.  gpsimd (SWDGE) only gets the small tensors.
        nc.sync.dma_start(out=wq_sb[:, 0:2, :], in_=wq_d[:, 0:2, :])
        nc.scalar.dma_start(out=xts[0][:, 0:2, :], in_=xt_d[0][:, 0:2, :])
        nc.sync.dma_start(out=wq_sb[:, 2:4, :], in_=wq_d[:, 2:4, :])
        nc.scalar.dma_start(out=xts[0][:, 2:4, :], in_=xt_d[0][:, 2:4, :])
        nc.sync.dma_start(out=xts[0][:, 4:8, :], in_=xt_d[0][:, 4:8, :])
        nc.scalar.dma_start(out=wq_sb[:, 4:8, :], in_=wq_d[:, 4:8, :])
        nc.sync.dma_start(out=wk_sb[:, 0:4, :], in_=wk_d[:, 0:4, :])
        nc.scalar.dma_start(out=wk_sb[:, 4:8, :], in_=wk_d[:, 4:8, :])
        nc.gpsimd.dma_start(out=eye2, in_=eye2_d)
        nc.gpsimd.dma_start(out=bq_sb, in_=bq_d)
        nc.gpsimd.dma_start(out=bk_sb, in_=bk_d)
        nc.gpsimd.dma_start(out=bvb_sb, in_=bvb_d)
        nc.gpsimd.dma_start(out=mask_sb, in_=mask_d)
        nc.sync.dma_start(out=wv_sb[:, 0:4, :], in_=wv_d[:, 0:4, :])
        nc.scalar.dma_start(out=wv_sb[:, 4:8, :], in_=wv_d[:, 4:8, :])
        nc.sync.dma_start(out=xts[1][:, 0:4, :], in_=xt_d[1][:, 0:4, :])
        nc.scalar.dma_start(out=xts[1][:, 4:8, :], in_=xt_d[1][:, 4:8, :])
        nc.sync.dma_start(out=xts[2][:, 0:4, :], in_=xt_d[2][:, 0:4, :])
        nc.scalar.dma_start(out=xts[2][:, 4:8, :], in_=xt_d[2][:, 4:8, :])
        nc.sync.dma_start(out=xts[3][:, 0:4, :], in_=xt_d[3][:, 0:4, :])
        nc.scalar.dma_start(out=xts[3][:, 4:8, :], in_=xt_d[3][:, 4:8, :])
        nc.sync.dma_start(out=wp_sb[:, 0:2, :], in_=wp_d[:, 0:2, :])
        nc.scalar.dma_start(out=wp_sb[:, 2:4, :], in_=wp_d[:, 2:4, :])

        # ones + pad columns of va are static: even heads carry ones at col
        # 64 (denominator -> PSUM row 64), odd heads at col 65 (-> row 65),
        # so the two denominators land on rows 64/65 of the two PV psums and
        # merge with a single DVE add
        va_hc = va.rearrange("p i (h c) -> p i h c", c=VW)
        va_pp = va.rearrange("p i (g w) -> p (i g) w", w=2 * VW)
        nc.vector.memset(va_pp[:, :, 64:65], 1.0)
        nc.vector.memset(va_pp[:, :, 65:66], 0.0)
        nc.vector.memset(va_pp[:, :, VW + 64 : VW + 65], 0.0)
        nc.vector.memset(va_pp[:, :, VW + 65 : VW + 66], 1.0)

        # ---------------- work-item emitters ----------------
        qk_state = {}

        def emit_qk_quarter(j, r, which, qi):
            """Quarter of a Q/K projection unit: 2 e-tile matmuls; the last
            quarter finishes the accumulation and adds the bias on DVE."""
            w_sb, dst, b_sb = (
                (wq_sb, qt, bq_sb) if which == "q" else (wk_sb, kt, bk_sb)
            )
            key = (j, r, which)
            if qi == 0:
                qk_state[key] = psp.tile(
                    [128, 512], f32, tag="mm512", bufs=2, name="qkps"
                )
            ps = qk_state[key]
            for et in range(2 * qi, 2 * qi + 2):
                nc.tensor.matmul(
                    ps,
                    lhsT=w_sb[:, et, r * 128 : (r + 1) * 128],
                    rhs=xts[j][:, et, :],
                    start=(et == 0),
                    stop=(et == ET - 1),
                )
            if qi == 3:
                del qk_state[key]
                sjl = slice(j * 512, (j + 1) * 512)
                nc.vector.tensor_scalar_add(
                    dst[:, r, sjl], ps, b_sb[:, r : r + 1]
                )

        def emit_v_quarter(j, ii, qi):
            i = 4 * j + ii
            si = slice(ii * 128, (ii + 1) * 128)
            key = ("v", j, ii)
            if qi == 0:
                qk_state[key] = psp.tile(
                    [128, 512], f32, tag="mm512", bufs=2, name="vps"
                )
            ps = qk_state[key]
            for et in range(2 * qi, 2 * qi + 2):
                nc.tensor.matmul(
                    ps,
                    lhsT=xts[j][:, et, si],
                    rhs=wv_sb[:, et, :],
                    start=(et == 0),
                    stop=(et == ET - 1),
                )
            if qi == 3:
                del qk_state[key]
                va_i = va_hc[:, i]
                nc.vector.tensor_tensor(
                    va_i[:, :, 0:64],
                    ps.rearrange("p (h d) -> p h d", d=64),
                    bvb_sb.rearrange("p (h d) -> p h d", d=64),
                    op=mybir.AluOpType.add,
                )

        attn_state = {}
        attn_pr = {}

        def emit_sc(j, r, ti):
            """Score pair for item ti + exp on ACT (+ causal mask on DVE for
            diagonal items).  PV is emitted separately, 2 items later."""
            if ti == 0:
                attn_state[(j, r)] = [
                    psp.tile([VW, 512], f32, tag=f"o{hh}", bufs=1,
                             name=f"outp{hh}")
                    for hh in range(2)
                ]
            tis = slice(ti * 128, (ti + 1) * 128)
            v = max(ti - 4 * j, 0)
            w = 512 - 128 * v
            sjv = slice(j * 512 + 128 * v, (j + 1) * 512)
            scp = psp.tile([128, 2, 512], f32, tag="sc", bufs=2)
            for hh in range(2):
                po = hh * 64
                nc.tensor.matmul(
                    scp[:, hh, 128 * v :],
                    lhsT=kt[po : po + 64, r, tis],
                    rhs=qt[po : po + 64, r, sjv],
                    start=True,
                    stop=True,
                )
            pr = work.tile([128, 2, 512], mdt, tag="pr", bufs=6)
            nc.scalar.activation(
                pr[:, :, 128 * v :], scp[:, :, 128 * v :], AF.Exp, scale=SCALE
            )
            if v or ti == 4 * j:
                # only the diagonal 128x128 sub-block needs masking: for
                # columns past it every t in this block satisfies t <= s
                dsl = slice(128 * v, 128 * v + 128)
                nc.vector.tensor_tensor(
                    pr[:, :, dsl], pr[:, :, dsl], mask_sb[:, v, :, dsl],
                    op=mybir.AluOpType.mult,
                )
            attn_pr[(j, r, ti)] = pr

        def emit_pv(j, r, ti):
            nt = 4 * j + 4
            outps = attn_state[(j, r)]
            pr = attn_pr.pop((j, r, ti))
            v = max(ti - 4 * j, 0)
            for hh in range(2):
                h = 2 * r + hh
                nc.tensor.matmul(
                    outps[hh][:, 128 * v :],
                    lhsT=va[:, ti, h * VW : (h + 1) * VW],
                    rhs=pr[:, hh, 128 * v :],
                    start=(ti == 0),
                    stop=(ti == nt - 1),
                )

        def emit_norm33_half(h):
            """Half-norm for the last pair (3,3): columns [256h, 256h+256).
            Half 0 is final after pv(13), so it (and its proj3b units) is
            emitted BEFORE pv(14)/pv(15) -- Tile serializes the psum-bank
            reads against the remaining PV writes, pipelining the tail."""
            outps = attn_state[(3, 3)]
            cs = slice(256 * h, 256 * h + 256)
            sjl = slice(3 * 512 + 256 * h, 3 * 512 + 256 * h + 256)
            osbd = work.tile([128, 512], mdt, tag="osbd", bufs=2)
            nc.vector.tensor_copy(osbd[64:66, cs], outps[1][64:66, cs])
            nc.vector.tensor_copy(osbd[64:65, cs], outps[0][64:65, cs])
            rdp = psp.tile([128, 512], f32, tag="sc", bufs=2, name="bcst")
            nc.tensor.matmul(
                rdp[:, cs],
                lhsT=eye2[64:66, :],
                rhs=osbd[64:66, cs],
                start=True,
                stop=True,
            )
            rd = work.tile([128, 512], f32, tag="rd", bufs=2)
            nc.vector.reciprocal_approx_fast(rd[:, cs], rdp[:, cs])
            nc.vector.tensor_mul(
                cat[0:64, 3, sjl], outps[0][0:64, cs], rd[0:64, cs]
            )
            nc.vector.tensor_mul(
                cat[64:128, 3, sjl], outps[1][0:64, cs], rd[64:128, cs]
            )
            if h == 1:
                attn_state.pop((3, 3))

        norm_state = {}

        def emit_norm_a(j, r):
            """Denominator copies (DVE only) -- emitted early so the bcst
            matmul in emit_norm_b never exposes the DVE latency on PE."""
            outps = attn_state[(j, r)]
            # both denominators to adjacent partitions 64/65, then ONE K=2
            # matmul broadcasts h0's to rows 0-63 and h1's to rows 64-127
            osbd = work.tile([128, 512], mdt, tag="osbd", bufs=2)
            nc.vector.tensor_copy(osbd[64:66, :], outps[1][64:66, :])
            nc.vector.tensor_copy(osbd[64:65, :], outps[0][64:65, :])
            norm_state[(j, r)] = osbd

        def emit_norm_b(j, r):
            outps = attn_state.pop((j, r))
            osbd = norm_state.pop((j, r))
            sjl = slice(j * 512, (j + 1) * 512)
            rdp = psp.tile([128, 512], f32, tag="sc", bufs=2, name="bcst")
            nc.tensor.matmul(
                rdp,
                lhsT=eye2[64:66, :],
                rhs=osbd[64:66, :],
                start=True,
                stop=True,
            )
            # normalize directly from the PV psum (mixed PSUM+SB inputs are
            # exempt from the equal-base-partition rule)
            rd = work.tile([128, 512], f32, tag="rd", bufs=2)
            nc.vector.reciprocal_approx_fast(rd, rdp)
            nc.vector.tensor_mul(
                cat[0:64, r, sjl], outps[0][0:64, :], rd[0:64, :]
            )
            nc.vector.tensor_mul(
                cat[64:128, r, sjl], outps[1][0:64, :], rd[64:128, :]
            )

        def emit_norm(j, r):
            emit_norm_a(j, r)
            emit_norm_b(j, r)

        def emit_proj_piece(j, sb, f, rr, on_act=False):
            """Output-projection piece: contraction pairs rr=(0,1) or (2,3).
            Tail-woven pieces evacuate psum on the (idle) scalar engine so
            the tail's DVE chain is untouched."""
            ss = slice(sb * 128, (sb + 1) * 128)
            sf = slice(f * 512, (f + 1) * 512)
            key = ("p", j, sb, f)
            if rr[0] == 0:
                qk_state[key] = psp.tile(
                    [128, 512], f32, tag="mm512", bufs=2, name="yproj"
                )
            yp = qk_state[key]
            for r in rr:
                nc.tensor.matmul(
                    yp,
                    lhsT=cat[:, r, ss],
                    rhs=wp_sb[:, r, sf],
                    start=(r == 0),
                    stop=(r == PAIRS - 1),
                )
            if rr[-1] == PAIRS - 1:
                del qk_state[key]
                ykey = ("ys", sb)
                if f == 0:
                    qk_state[ykey] = work.tile(
                        [128, 2, 512], mdt, tag="ys", bufs=2, name="yspair"
                    )
                ys = qk_state[ykey]
                if on_act:
                    nc.scalar.copy(ys[:, f, :], yp)
                else:
                    nc.vector.tensor_copy(ys[:, f, :], yp)
                if f == 1:
                    del qk_state[ykey]
                    nc.sync.dma_start(out=y_d[ss, :], in_=ys)

        def emit_proj3_a(u):
            """Chunk-3 proj unit u (sb=12+u//2, f=u%2): r0+r1 into stash."""
            sb, f = 12 + u // 2, u % 2
            ss = slice(sb * 128, (sb + 1) * 128)
            sf = slice(f * 512, (f + 1) * 512)
            yp = psp.tile([128, 512], f32, tag="mm512", bufs=2, name="yp3a")
            for r in (0, 1):
                nc.tensor.matmul(
                    yp,
                    lhsT=cat[:, r, ss],
                    rhs=wp_sb[:, r, sf],
                    start=(r == 0),
                    stop=(r == 1),
                )
            nc.vector.tensor_copy(y01[:, u, :], yp)

        def emit_proj3_c(u):
            """Chunk-3 proj unit u: fold r2 into the stash after norm(3,2)."""
            sb, f = 12 + u // 2, u % 2
            ss = slice(sb * 128, (sb + 1) * 128)
            sf = slice(f * 512, (f + 1) * 512)
            yp = psp.tile([128, 512], f32, tag="mm512", bufs=2, name="yp3c")
            nc.tensor.matmul(
                yp, lhsT=cat[:, 2, ss], rhs=wp_sb[:, 2, sf], start=True,
                stop=True,
            )
            nc.vector.tensor_tensor(
                y01[:, u, :], y01[:, u, :], yp, op=mybir.AluOpType.add
            )

        def emit_proj3_b(u):
            """Chunk-3 proj unit u: r3 only, add stash, emit output."""
            sb, f = 12 + u // 2, u % 2
            ss = slice(sb * 128, (sb + 1) * 128)
            sf = slice(f * 512, (f + 1) * 512)
            yp = psp.tile([128, 512], f32, tag="mm512", bufs=2, name="yp3b")
            nc.tensor.matmul(
                yp, lhsT=cat[:, 3, ss], rhs=wp_sb[:, 3, sf], start=True,
                stop=True,
            )
            ykey = ("ys3", sb)
            if f == 0:
                qk_state[ykey] = work.tile(
                    [128, 2, 512], mdt, tag="ys", bufs=2, name="yspair3"
                )
            ys = qk_state[ykey]
            nc.vector.tensor_tensor(
                ys[:, f, :], yp, y01[:, u, :], op=mybir.AluOpType.add
            )
            if f == 1:
                del qk_state[ykey]
                nc.sync.dma_start(out=y_d[ss, :], in_=ys)

        # ---------------- need-weighted interleaved emission ----------------
        def interleave(main, filler):
            """main/filler: lists of (weight, fn). Filler cost is consumed
            proportionally to accumulated main weight, so segments with more
            dependency latency (diagonal blocks, norms) get denser filler."""
            tm = sum(c for c, _ in main) or 1
            tf = sum(c for c, _ in filler)
            rho = tf / tm
            fs = 0.0
            ms = 0.0
            fi = 0
            for c, fn in main:
                fn()
                ms += c
                while fi < len(filler) and fs + filler[fi][0] * 0.5 <= ms * rho:
                    fs += filler[fi][0]
                    filler[fi][1]()
                    fi += 1
            while fi < len(filler):
                filler[fi][1]()
                fi += 1
        CQK = 440  # filler piece cost: 2 N=512 matmuls
        CPROJ = 450  # filler piece cost: 2 N=512 matmuls
        CNORM = 1300
        CNORMA = 200

        def w_sc(v):
            # per-item period is exp-paced: (2w+352)/1.2 minus the PV share
            w = 512 - 128 * v
            return 0.833 * w + 283

        def w_pv(v):
            w = 512 - 128 * v
            return 0.833 * w + 10

        def pipe(j, r):
            """Software-pipelined attention item list for one pair: PV lags
            scores by 2 items so the PE FIFO never blocks on the exp.  The
            norm is NOT included -- it is spliced ~4 items into the next
            pair's stream so its psum slot ("sc" pool) and DVE chain never
            block the next pair's score/exp cadence."""
            nt = 4 * j + 4
            seq = []
            for ti in range(nt):
                v = max(ti - 4 * j, 0)
                seq.append((w_sc(v), lambda j=j, r=r, ti=ti: emit_sc(j, r, ti)))
                if ti >= 2:
                    pv_ti = ti - 2
                    pvv = max(pv_ti - 4 * j, 0)
                    seq.append(
                        (w_pv(pvv), lambda j=j, r=r, t=pv_ti: emit_pv(j, r, t))
                    )
            for pv_ti in (nt - 2, nt - 1):
                pvv = max(pv_ti - 4 * j, 0)
                wt = w_pv(pvv) + (400 if pv_ti == nt - 1 else 0)
                seq.append((wt, lambda j=j, r=r, t=pv_ti: emit_pv(j, r, t)))
            return seq

        def qk_pieces(j, r, which):
            return [
                (CQK, lambda j=j, r=r, w=which, q=q: emit_qk_quarter(j, r, w, q))
                for q in range(4)
            ]

        def v_pieces(j, ii):
            return [
                (CQK, lambda j=j, ii=ii, q=q: emit_v_quarter(j, ii, q))
                for q in range(4)
            ]

        def proj_pieces(j, sb, f):
            return [
                (CPROJ, lambda j=j, sb=sb, f=f, rr=rr: emit_proj_piece(j, sb, f, rr))
                for rr in ((0, 1), (2, 3))
            ]

        # chunk 0: Q first, ordered so compute starts when the first half
        # of wq and xts[0] has landed, while keeping at most two open
        # accumulation groups (mm512 pool is bufs=2)
        for r0, r1 in ((0, 1), (2, 3)):
            for qi in range(4):
                emit_qk_quarter(0, r0, "q", qi)
                emit_qk_quarter(0, r1, "q", qi)

        norm_carry = None  # deferred norm of the previous pair/chunk
        for j in range(SCH):
            # K(j)/V(j) must precede chunk-j diagonal blocks in program
            # order.  Chunk 0: r0's attention is zipped directly into the
            # K/V stream so ACT starts as soon as K(r0)/V(0..3) land.
            # Chunk 1: K(1)/V(1) already ran as chunk-0 filler, so phase 1
            # is just the non-diag pipeline.  Chunks 2-3: zip K/V into r0's
            # non-diagonal pipeline as before.
            if j == 0:
                for q in range(4):
                    emit_qk_quarter(0, 0, "k", q)
                for q in range(4):
                    emit_qk_quarter(0, 1, "k", q)
                emit_sc(0, 0, 0)
                emit_sc(0, 0, 1)
                for q in range(4):
                    emit_v_quarter(0, 0, q)
                for q in range(4):
                    emit_v_quarter(0, 1, q)
                emit_sc(0, 0, 2)
                emit_sc(0, 0, 3)
                emit_pv(0, 0, 0)
                emit_pv(0, 0, 1)
                for q in range(4):
                    emit_v_quarter(0, 2, q)
                for q in range(4):
                    emit_v_quarter(0, 3, q)
                emit_pv(0, 0, 2)
                emit_pv(0, 0, 3)
                for q in range(4):
                    emit_qk_quarter(0, 2, "k", q)
                for q in range(4):
                    emit_qk_quarter(0, 3, "k", q)
                rest0 = []
            else:
                p0 = pipe(j, 0)
                # entries per sc(ti): 1 for ti<2, else 2 (sc + lagged pv)
                ndlen = 4 * j + max(0, 4 * j - 2)
                nd0, rest0 = p0[:ndlen], p0[ndlen:]
                if norm_carry is not None:
                    nd0.insert(min(3, len(nd0)), norm_carry)
                    norm_carry = None
                kv = []
                for rp in ((0, 1), (2, 3)):
                    for qi in range(4):
                        kv.append(qk_pieces(j, rp[0], "k")[qi])
                        kv.append(qk_pieces(j, rp[1], "k")[qi])
                for ip in ((0, 1), (2, 3)):
                    for qi in range(4):
                        kv.append(v_pieces(j, ip[0])[qi])
                        kv.append(v_pieces(j, ip[1])[qi])
                interleave(nd0, kv)
            # rest: r0 diagonal, then r1..r3; each pair's norm is spliced
            # ~4 entries into the NEXT pair's stream
            main = list(rest0)
            for r in range(1, PAIRS):
                pr_items = pipe(j, r)
                if j == 3 and r == 3:
                    # drop the trailing pv(14)/pv(15): re-emitted below
                    # around the half-norms
                    pr_items = pr_items[:-2]
                pr_items.insert(
                    min(4, len(pr_items)),
                    (CNORM, lambda j=j, r=r - 1: emit_norm(j, r)),
                )
                if j == 3 and r == 2:
                    # after norm(3,1) both r0/r1 cat chunks exist: stash
                    # r0+r1 proj partials for the final s-chunk
                    for u in range(8):
                        pr_items.insert(
                            5 + u, (CPROJ, lambda u=u: emit_proj3_a(u))
                        )
                if j == 3 and r == 3:
                    # fold r2 into the stash so only r3 remains at the tail
                    for u in range(8):
                        pr_items.insert(
                            5 + u, (CPROJ, lambda u=u: emit_proj3_c(u))
                        )
                    # pipelined tail: half-norm 0 + its proj3b units run
                    # before pv(14)/pv(15); then half 1 + the rest
                    pr_items.append((CNORM, lambda: emit_norm33_half(0)))
                    for u in range(4):
                        pr_items.append(
                            (0, lambda u=u: emit_proj3_b(u))
                        )
                    for t in (14, 15):
                        pr_items.append(
                            (0, lambda t=t: emit_pv(3, 3, t))
                        )
                    pr_items.append((0, lambda: emit_norm33_half(1)))
                    for u in range(4, 8):
                        pr_items.append(
                            (0, lambda u=u: emit_proj3_b(u))
                        )
                main += pr_items
            if j + 1 < SCH:
                norm_carry = (CNORM, lambda j=j: emit_norm(j, PAIRS - 1))
            filler = []
            if j + 1 < SCH:
                for rp in ((0, 1), (2, 3)):
                    for qi in range(4):
                        filler.append(qk_pieces(j + 1, rp[0], "q")[qi])
                        filler.append(qk_pieces(j + 1, rp[1], "q")[qi])
            if j == 2:
                for sb in range(0, 6):
                    for f in range(2):
                        filler += proj_pieces(sb // 4, sb, f)
            if j == 3:
                for sb in range(6, 12):
                    for f in range(2):
                        filler += proj_pieces(sb // 4, sb, f)
            interleave(main, filler)

    nc.compile()
    return nc


def get_nc():
    if "nc" not in _CACHE:
        _CACHE["nc"] = _build()
    return _CACHE["nc"]


def prep_core_inputs(x, Wq, bq, Wk, bk, Wv, bv, Wp, core):
    """Pack the full-model inputs into one core's input map."""
    b, g = core // 2, core % 2
    heads = list(range(g * 8, g * 8 + 8))

    def pack_w(W):  # [H,E,D] -> local [E, 512] -> [128, 8, 512]
        Wl = np.concatenate([W[h] for h in heads], axis=1)
        return host_round(Wl.reshape(ET, 128, 512).transpose(1, 0, 2))

    wp_l = host_round(
        Wp[g * 512 : (g + 1) * 512].reshape(PAIRS, 128, E).transpose(1, 0, 2)
    )

    def pack_b(bias):
        return np.stack(
            [
                np.concatenate([bias[heads[2 * r]], bias[heads[2 * r + 1]]])
                for r in range(PAIRS)
            ],
            axis=1,
        ).astype(np.float32)

    bv_cat = np.concatenate([bv[h] for h in heads]).astype(np.float32)

    e2 = np.zeros((128, 128), np.float32)
    e2[64, 0:64] = 1.0
    e2[65, 64:128] = 1.0

    p = np.arange(128)[:, None, None]
    v = np.arange(4)[None, :, None]
    c = np.arange(512)[None, None, :]
    mask = (c >= p + 128 * v).astype(np.float32)  # [128, 4, 512]
    mask = host_round(np.repeat(mask[:, :, None, :], 2, axis=2))

    xe = x[b].T.reshape(ET, 128, SCH, 512)  # [et, p, j, s]
    return {
        "xt": host_round(np.ascontiguousarray(xe.transpose(2, 1, 0, 3))),
        "wq": pack_w(Wq),
        "wk": pack_w(Wk),
        "wv": pack_w(Wv),
        "wp": wp_l,
        "bq": pack_b(bq),
        "bk": pack_b(bk),
        "bvb": np.tile(bv_cat[None, :], (128, 1)),
        "mask": mask,
        "eye2": host_round(e2),
    }


def kernel(**inputs):
    from concourse.bass_utils import run_bass_kernel_spmd

    args = {k: np.asarray(v, np.float32) for k, v in inputs.items()}
    nc = get_nc()
    in_maps = [
        prep_core_inputs(
            args["x"], args["Wq"], args["bq"], args["Wk"], args["bk"],
            args["Wv"], args["bv"], args["Wp"], c,
        )
        for c in range(NCORES)
    ]
    res = run_bass_kernel_spmd(nc, in_maps, core_ids=list(range(NCORES)))
    parts = [np.asarray(r["y"], np.float32) for r in res.results]
    out = np.stack([parts[2 * b] + parts[2 * b + 1] for b in range(B)])
    return (out + args["bp"][None, None, :]).astype(np.float32)


# revision 53
# speedup vs baseline: 1.0292x; 1.0075x over previous
"""Multi-head causal attention (B=4, S=2048, E=1024, H=16, D=64) on 8 trn2 cores.

Sharding: core c handles batch b = c//2 and head-group g = c%2 (8 heads each).
Each core computes its partial output projection over its 512 local concat
columns; the host sums the two partials per batch and adds bp.

Layout strategy (per core):
  - x is pre-transposed on host: xT [1024(+pad), S] as [128, 8, S] e-tiles.
  - Q^T, K^T computed as [d, s] (d on partitions, 2 heads per 128-partition
    pair tile) so scores come out transposed: scoresT [t, s].  The two heads
    of a pair sit at partitions 0-63 / 64-127, so the score matmuls of a pair
    run concurrently on different PE row groups.
  - V kept natural [t, d] with a ones column per head (66-col stride), so the
    PV matmul also produces the softmax denominator as row 64/65 of its
    output.  V bias is added on DVE from a host-broadcast tile.
  - Softmax: exp on ACT (no max subtraction -- scores are O(1) by
    construction), causal masking via ONE broadcast multiplicative 0/1 mask
    on DVE per diag item, both denominators merged to rows 64/65 with a
    single DVE add, broadcast across partitions via a K=2 matmul,
    reciprocal on DVE, normalize into concat^T, output projection from
    concat^T, y output in bf16 (host does f32 sum).
  - Attention is software-pipelined: the PV matmuls of item ti are emitted
    after the score matmuls of item ti+2, so the PE FIFO never head-blocks
    on the ACT exp of the current item (exp latency ~1.15us/item is the
    pacer in late chunks; the 2-item lag lets scores+fillers run under it).
  - Scheduling: attention items are interleaved with "filler" pieces (2
    matmuls each) from the QKV projections of later chunks and the output
    projection of earlier chunks, spread by estimated cost so the tensor
    engine never starves while ACT/DVE work through the softmax chain.
  - Head: first DMAs are split into 256KB et-pair slices across both HWDGE
    queues so the first Q matmuls start ~9us instead of ~14us; the 2MB wp
    (not needed until chunk 2) is deferred to the queue tails; y outputs go
    out on the HWDGE queues (idle after input load) instead of SWDGE.
"""

import numpy as np

B, S, E, H, D = 4, 2048, 1024, 16, 64
NCORES = 8
PAIRS = 4  # head pairs per core (8 heads)
ET = 8  # e-tiles of 128 for the contraction over E
SCH = 4  # s-chunks of 512
VW = 66  # V columns per head: 64 d + 1 ones + 1 pad
SCALE = float(D) ** -0.5

MM_DTYPE = "bfloat16"

_CACHE = {}


def host_round(a):
    import ml_dtypes

    return np.ascontiguousarray(a, np.float32).astype(ml_dtypes.bfloat16)


def _build():
    import concourse.tile as tile
    from concourse import bacc, mybir
    from contextlib import ExitStack

    f32 = mybir.dt.float32
    bf16 = mybir.dt.bfloat16
    mdt = getattr(mybir.dt, MM_DTYPE)
    AF = mybir.ActivationFunctionType

    nc = bacc.Bacc("TRN2", target_bir_lowering=False, debug=False, num_devices=NCORES)

    xt_d = nc.dram_tensor("xt", [SCH, 128, ET, 512], mdt, kind="ExternalInput").ap()
    wq_d = nc.dram_tensor("wq", [128, ET, 512], mdt, kind="ExternalInput").ap()
    wk_d = nc.dram_tensor("wk", [128, ET, 512], mdt, kind="ExternalInput").ap()
    wv_d = nc.dram_tensor("wv", [128, ET, 512], mdt, kind="ExternalInput").ap()
    wp_d = nc.dram_tensor("wp", [128, PAIRS, E], mdt, kind="ExternalInput").ap()
    bq_d = nc.dram_tensor("bq", [128, PAIRS], f32, kind="ExternalInput").ap()
    bk_d = nc.dram_tensor("bk", [128, PAIRS], f32, kind="ExternalInput").ap()
    bvb_d = nc.dram_tensor("bvb", [128, 512], f32, kind="ExternalInput").ap()
    mask_d = nc.dram_tensor("mask", [128, 4, 2, 512], mdt, kind="ExternalInput").ap()
    eye2_d = nc.dram_tensor("eye2", [128, 128], mdt, kind="ExternalInput").ap()
    eye128_d = nc.dram_tensor("eye128", [128, 128], mdt, kind="ExternalInput").ap()
    y_d = nc.dram_tensor("y", [S, E], bf16, kind="ExternalOutput").ap()

    with tile.TileContext(nc) as tc, ExitStack() as ctx:
        pers = ctx.enter_context(tc.tile_pool(name="pers", bufs=1))
        work = ctx.enter_context(tc.tile_pool(name="work", bufs=1))
        psp = ctx.enter_context(tc.tile_pool(name="psp", bufs=1, space="PSUM"))

        qt = pers.tile([128, PAIRS, S], mdt)  # Q^T pair tiles
        kt = pers.tile([128, PAIRS, S], mdt)  # K^T pair tiles
        va = pers.tile([128, 16, 8 * VW], mdt)  # V (+ones col) per t-block
        cat = pers.tile([128, PAIRS, S], mdt)  # concat^T
        bq_sb = pers.tile([128, PAIRS], f32)
        bk_sb = pers.tile([128, PAIRS], f32)
        bvb_sb = pers.tile([128, 512], f32)
        eye2 = pers.tile([128, 128], mdt)  # bc lhsT: rows 64/65 block-select
        eye128 = pers.tile([128, 128], mdt)  # identity: psum pre-seed matmul
        wq_sb = pers.tile([128, ET, 512], mdt)
        wk_sb = pers.tile([128, ET, 512], mdt)
        wv_sb = pers.tile([128, ET, 512], mdt)
        wp_sb = pers.tile([128, PAIRS, E], mdt)
        mask_sb = pers.tile([128, 4, 2, 512], mdt)  # mask duplicated per head
        y01 = pers.tile([128, 8, 512], mdt)  # chunk-3 proj r0+r1 partials
        xts = [
            work.tile([128, ET, 512], mdt, tag=f"xt{j % 2}", name=f"xt{j}")
            for j in range(SCH)
        ]

        # ---- head DMAs: only sync/scalar (HWDGE) + gpsimd (SWDGE) can issue.
        # First-needed data goes first in 256KB et-pair slices alternating
        # across the two HWDGE queues so Q(0) compute can start as soon as
        # the first pair lands.  wp (2MB, needed only from chunk 2) is
        # deferred to the queue tails so it doesn't steal HBM bandwidth from
        # the critical path.  gpsimd (SWDGE) only gets the small tensors.
        nc.sync.dma_start(out=wq_sb[:, 0:2, :], in_=wq_d[:, 0:2, :])
        nc.scalar.dma_start(out=xts[0][:, 0:2, :], in_=xt_d[0][:, 0:2, :])
        nc.sync.dma_start(out=wq_sb[:, 2:4, :], in_=wq_d[:, 2:4, :])
        nc.scalar.dma_start(out=xts[0][:, 2:4, :], in_=xt_d[0][:, 2:4, :])
        nc.sync.dma_start(out=xts[0][:, 4:8, :], in_=xt_d[0][:, 4:8, :])
        nc.scalar.dma_start(out=wq_sb[:, 4:8, :], in_=wq_d[:, 4:8, :])
        nc.sync.dma_start(out=wk_sb[:, 0:4, :], in_=wk_d[:, 0:4, :])
        nc.scalar.dma_start(out=wk_sb[:, 4:8, :], in_=wk_d[:, 4:8, :])
        nc.gpsimd.dma_start(out=eye2, in_=eye2_d)
        nc.gpsimd.dma_start(out=eye128, in_=eye128_d)
        nc.gpsimd.dma_start(out=bq_sb, in_=bq_d)
        nc.gpsimd.dma_start(out=bk_sb, in_=bk_d)
        nc.gpsimd.dma_start(out=bvb_sb, in_=bvb_d)
        nc.gpsimd.dma_start(out=mask_sb, in_=mask_d)
        nc.sync.dma_start(out=wv_sb[:, 0:4, :], in_=wv_d[:, 0:4, :])
        nc.scalar.dma_start(out=wv_sb[:, 4:8, :], in_=wv_d[:, 4:8, :])
        nc.sync.dma_start(out=xts[1][:, 0:4, :], in_=xt_d[1][:, 0:4, :])
        nc.scalar.dma_start(out=xts[1][:, 4:8, :], in_=xt_d[1][:, 4:8, :])
        nc.sync.dma_start(out=xts[2][:, 0:4, :], in_=xt_d[2][:, 0:4, :])
        nc.scalar.dma_start(out=xts[2][:, 4:8, :], in_=xt_d[2][:, 4:8, :])
        nc.sync.dma_start(out=xts[3][:, 0:4, :], in_=xt_d[3][:, 0:4, :])
        nc.scalar.dma_start(out=xts[3][:, 4:8, :], in_=xt_d[3][:, 4:8, :])
        nc.sync.dma_start(out=wp_sb[:, 0:2, :], in_=wp_d[:, 0:2, :])
        nc.scalar.dma_start(out=wp_sb[:, 2:4, :], in_=wp_d[:, 2:4, :])

        # ones + pad columns of va are static: even heads carry ones at col
        # 64 (denominator -> PSUM row 64), odd heads at col 65 (-> row 65),
        # so the two denominators land on rows 64/65 of the two PV psums and
        # merge with a single DVE add
        va_hc = va.rearrange("p i (h c) -> p i h c", c=VW)
        va_pp = va.rearrange("p i (g w) -> p (i g) w", w=2 * VW)
        nc.vector.memset(va_pp[:, :, 64:65], 1.0)
        nc.vector.memset(va_pp[:, :, 65:66], 0.0)
        nc.vector.memset(va_pp[:, :, VW + 64 : VW + 65], 0.0)
        nc.vector.memset(va_pp[:, :, VW + 65 : VW + 66], 1.0)

        # ---------------- work-item emitters ----------------
        qk_state = {}

        def emit_qk_quarter(j, r, which, qi):
            """Quarter of a Q/K projection unit: 2 e-tile matmuls; the last
            quarter finishes the accumulation and adds the bias on DVE."""
            w_sb, dst, b_sb = (
                (wq_sb, qt, bq_sb) if which == "q" else (wk_sb, kt, bk_sb)
            )
            key = (j, r, which)
            if qi == 0:
                qk_state[key] = psp.tile(
                    [128, 512], f32, tag="mm512", bufs=2, name="qkps"
                )
            ps = qk_state[key]
            for et in range(2 * qi, 2 * qi + 2):
                nc.tensor.matmul(
                    ps,
                    lhsT=w_sb[:, et, r * 128 : (r + 1) * 128],
                    rhs=xts[j][:, et, :],
                    start=(et == 0),
                    stop=(et == ET - 1),
                )
            if qi == 3:
                del qk_state[key]
                sjl = slice(j * 512, (j + 1) * 512)
                nc.vector.tensor_scalar_add(
                    dst[:, r, sjl], ps, b_sb[:, r : r + 1]
                )

        def emit_v_quarter(j, ii, qi):
            i = 4 * j + ii
            si = slice(ii * 128, (ii + 1) * 128)
            key = ("v", j, ii)
            if qi == 0:
                qk_state[key] = psp.tile(
                    [128, 512], f32, tag="mm512", bufs=2, name="vps"
                )
            ps = qk_state[key]
            for et in range(2 * qi, 2 * qi + 2):
                nc.tensor.matmul(
                    ps,
                    lhsT=xts[j][:, et, si],
                    rhs=wv_sb[:, et, :],
                    start=(et == 0),
                    stop=(et == ET - 1),
                )
            if qi == 3:
                del qk_state[key]
                va_i = va_hc[:, i]
                nc.vector.tensor_tensor(
                    va_i[:, :, 0:64],
                    ps.rearrange("p (h d) -> p h d", d=64),
                    bvb_sb.rearrange("p (h d) -> p h d", d=64),
                    op=mybir.AluOpType.add,
                )

        attn_state = {}
        attn_pr = {}

        def emit_sc(j, r, ti):
            """Score pair for item ti + exp on ACT (+ causal mask on DVE for
            diagonal items).  PV is emitted separately, 2 items later."""
            if ti == 0:
                attn_state[(j, r)] = [
                    psp.tile([VW, 512], f32, tag=f"o{hh}", bufs=1,
                             name=f"outp{hh}")
                    for hh in range(2)
                ]
            tis = slice(ti * 128, (ti + 1) * 128)
            v = max(ti - 4 * j, 0)
            w = 512 - 128 * v
            sjv = slice(j * 512 + 128 * v, (j + 1) * 512)
            scp = psp.tile([128, 2, 512], f32, tag="sc", bufs=2)
            for hh in range(2):
                po = hh * 64
                nc.tensor.matmul(
                    scp[:, hh, 128 * v :],
                    lhsT=kt[po : po + 64, r, tis],
                    rhs=qt[po : po + 64, r, sjv],
                    start=True,
                    stop=True,
                )
            pr = work.tile([128, 2, 512], mdt, tag="pr", bufs=6)
            nc.scalar.activation(
                pr[:, :, 128 * v :], scp[:, :, 128 * v :], AF.Exp, scale=SCALE
            )
            if v or ti == 4 * j:
                # only the diagonal 128x128 sub-block needs masking: for
                # columns past it every t in this block satisfies t <= s
                dsl = slice(128 * v, 128 * v + 128)
                nc.vector.tensor_tensor(
                    pr[:, :, dsl], pr[:, :, dsl], mask_sb[:, v, :, dsl],
                    op=mybir.AluOpType.mult,
                )
            attn_pr[(j, r, ti)] = pr

        def emit_pv(j, r, ti):
            nt = 4 * j + 4
            outps = attn_state[(j, r)]
            pr = attn_pr.pop((j, r, ti))
            v = max(ti - 4 * j, 0)
            for hh in range(2):
                h = 2 * r + hh
                nc.tensor.matmul(
                    outps[hh][:, 128 * v :],
                    lhsT=va[:, ti, h * VW : (h + 1) * VW],
                    rhs=pr[:, hh, 128 * v :],
                    start=(ti == 0),
                    stop=(ti == nt - 1),
                )

        def emit_norm33_half(h):
            """Half-norm for the last pair (3,3): columns [256h, 256h+256).
            Half 0 is final after pv(13), so it (and its proj3b units) is
            emitted BEFORE pv(14)/pv(15) -- Tile serializes the psum-bank
            reads against the remaining PV writes, pipelining the tail."""
            outps = attn_state[(3, 3)]
            cs = slice(256 * h, 256 * h + 256)
            sjl = slice(3 * 512 + 256 * h, 3 * 512 + 256 * h + 256)
            osbd = work.tile([128, 512], mdt, tag="osbd", bufs=2)
            nc.vector.tensor_copy(osbd[64:66, cs], outps[1][64:66, cs])
            nc.vector.tensor_copy(osbd[64:65, cs], outps[0][64:65, cs])
            rdp = psp.tile([128, 512], f32, tag="sc", bufs=2, name="bcst")
            nc.tensor.matmul(
                rdp[:, cs],
                lhsT=eye2[64:66, :],
                rhs=osbd[64:66, cs],
                start=True,
                stop=True,
            )
            rd = work.tile([128, 512], f32, tag="rd", bufs=2)
            nc.vector.reciprocal_approx_fast(rd[:, cs], rdp[:, cs])
            nc.vector.tensor_mul(
                cat[0:64, 3, sjl], outps[0][0:64, cs], rd[0:64, cs]
            )
            nc.vector.tensor_mul(
                cat[64:128, 3, sjl], outps[1][0:64, cs], rd[64:128, cs]
            )
            if h == 1:
                attn_state.pop((3, 3))

        norm_state = {}

        def emit_norm_a(j, r):
            """Denominator copies (DVE only) -- emitted early so the bcst
            matmul in emit_norm_b never exposes the DVE latency on PE."""
            outps = attn_state[(j, r)]
            # both denominators to adjacent partitions 64/65, then ONE K=2
            # matmul broadcasts h0's to rows 0-63 and h1's to rows 64-127
            osbd = work.tile([128, 512], mdt, tag="osbd", bufs=2)
            nc.vector.tensor_copy(osbd[64:66, :], outps[1][64:66, :])
            nc.vector.tensor_copy(osbd[64:65, :], outps[0][64:65, :])
            norm_state[(j, r)] = osbd

        def emit_norm_b(j, r):
            outps = attn_state.pop((j, r))
            osbd = norm_state.pop((j, r))
            sjl = slice(j * 512, (j + 1) * 512)
            rdp = psp.tile([128, 512], f32, tag="sc", bufs=2, name="bcst")
            nc.tensor.matmul(
                rdp,
                lhsT=eye2[64:66, :],
                rhs=osbd[64:66, :],
                start=True,
                stop=True,
            )
            # normalize directly from the PV psum (mixed PSUM+SB inputs are
            # exempt from the equal-base-partition rule)
            rd = work.tile([128, 512], f32, tag="rd", bufs=2)
            nc.vector.reciprocal_approx_fast(rd, rdp)
            nc.vector.tensor_mul(
                cat[0:64, r, sjl], outps[0][0:64, :], rd[0:64, :]
            )
            nc.vector.tensor_mul(
                cat[64:128, r, sjl], outps[1][0:64, :], rd[64:128, :]
            )

        def emit_norm(j, r):
            emit_norm_a(j, r)
            emit_norm_b(j, r)

        def emit_proj_piece(j, sb, f, rr, on_act=False):
            """Output-projection piece: contraction pairs rr=(0,1) or (2,3).
            Tail-woven pieces evacuate psum on the (idle) scalar engine so
            the tail's DVE chain is untouched."""
            ss = slice(sb * 128, (sb + 1) * 128)
            sf = slice(f * 512, (f + 1) * 512)
            key = ("p", j, sb, f)
            if rr[0] == 0:
                qk_state[key] = psp.tile(
                    [128, 512], f32, tag="mm512", bufs=2, name="yproj"
                )
            yp = qk_state[key]
            for r in rr:
                nc.tensor.matmul(
                    yp,
                    lhsT=cat[:, r, ss],
                    rhs=wp_sb[:, r, sf],
                    start=(r == 0),
                    stop=(r == PAIRS - 1),
                )
            if rr[-1] == PAIRS - 1:
                del qk_state[key]
                ykey = ("ys", sb)
                if f == 0:
                    qk_state[ykey] = work.tile(
                        [128, 2, 512], mdt, tag="ys", bufs=2, name="yspair"
                    )
                ys = qk_state[ykey]
                if on_act:
                    nc.scalar.copy(ys[:, f, :], yp)
                else:
                    nc.vector.tensor_copy(ys[:, f, :], yp)
                if f == 1:
                    del qk_state[ykey]
                    nc.sync.dma_start(out=y_d[ss, :], in_=ys)

        def emit_proj3_a(u):
            """Chunk-3 proj unit u (sb=12+u//2, f=u%2): r0+r1 into stash."""
            sb, f = 12 + u // 2, u % 2
            ss = slice(sb * 128, (sb + 1) * 128)
            sf = slice(f * 512, (f + 1) * 512)
            yp = psp.tile([128, 512], f32, tag="mm512", bufs=2, name="yp3a")
            for r in (0, 1):
                nc.tensor.matmul(
                    yp,
                    lhsT=cat[:, r, ss],
                    rhs=wp_sb[:, r, sf],
                    start=(r == 0),
                    stop=(r == 1),
                )
            nc.vector.tensor_copy(y01[:, u, :], yp)

        def emit_proj3_c(u):
            """Chunk-3 proj unit u: fold r2 into the stash after norm(3,2)."""
            sb, f = 12 + u // 2, u % 2
            ss = slice(sb * 128, (sb + 1) * 128)
            sf = slice(f * 512, (f + 1) * 512)
            yp = psp.tile([128, 512], f32, tag="mm512", bufs=2, name="yp3c")
            nc.tensor.matmul(
                yp, lhsT=cat[:, 2, ss], rhs=wp_sb[:, 2, sf], start=True,
                stop=True,
            )
            nc.vector.tensor_tensor(
                y01[:, u, :], y01[:, u, :], yp, op=mybir.AluOpType.add
            )

        def emit_proj3_b(u):
            """Chunk-3 proj unit u: pre-seed psum with the y01 stash via an
            identity matmul (PE is idle in the tail), accumulate r3 on top,
            evacuate on the (idle) scalar engine -- no DVE in the tail."""
            sb, f = 12 + u // 2, u % 2
            ss = slice(sb * 128, (sb + 1) * 128)
            sf = slice(f * 512, (f + 1) * 512)
            yp = psp.tile([128, 512], f32, tag="mm512", bufs=2, name="yp3b")
            nc.tensor.matmul(
                yp, lhsT=eye128, rhs=y01[:, u, :], start=True, stop=False
            )
            nc.tensor.matmul(
                yp, lhsT=cat[:, 3, ss], rhs=wp_sb[:, 3, sf], start=False,
                stop=True,
            )
            ykey = ("ys3", sb)
            if f == 0:
                qk_state[ykey] = work.tile(
                    [128, 2, 512], mdt, tag="ys", bufs=2, name="yspair3"
                )
            ys = qk_state[ykey]
            nc.scalar.copy(ys[:, f, :], yp)
            if f == 1:
                del qk_state[ykey]
                nc.sync.dma_start(out=y_d[ss, :], in_=ys)

        # ---------------- need-weighted interleaved emission ----------------
        def interleave(main, filler):
            """main/filler: lists of (weight, fn). Filler cost is consumed
            proportionally to accumulated main weight, so segments with more
            dependency latency (diagonal blocks, norms) get denser filler."""
            tm = sum(c for c, _ in main) or 1
            tf = sum(c for c, _ in filler)
            rho = tf / tm
            fs = 0.0
            ms = 0.0
            fi = 0
            for c, fn in main:
                fn()
                ms += c
                while fi < len(filler) and fs + filler[fi][0] * 0.5 <= ms * rho:
                    fs += filler[fi][0]
                    filler[fi][1]()
                    fi += 1
            while fi < len(filler):
                filler[fi][1]()
                fi += 1
        CQK = 440  # filler piece cost: 2 N=512 matmuls
        CPROJ = 450  # filler piece cost: 2 N=512 matmuls
        CNORM = 1300
        CNORMA = 200

        def w_sc(v):
            # per-item period is exp-paced: (2w+352)/1.2 minus the PV share
            w = 512 - 128 * v
            return 0.833 * w + 283

        def w_pv(v):
            w = 512 - 128 * v
            return 0.833 * w + 10

        def pipe(j, r):
            """Software-pipelined attention item list for one pair: PV lags
            scores by 2 items so the PE FIFO never blocks on the exp.  The
            norm is NOT included -- it is spliced ~4 items into the next
            pair's stream so its psum slot ("sc" pool) and DVE chain never
            block the next pair's score/exp cadence."""
            nt = 4 * j + 4
            seq = []
            for ti in range(nt):
                v = max(ti - 4 * j, 0)
                seq.append((w_sc(v), lambda j=j, r=r, ti=ti: emit_sc(j, r, ti)))
                if ti >= 2:
                    pv_ti = ti - 2
                    pvv = max(pv_ti - 4 * j, 0)
                    seq.append(
                        (w_pv(pvv), lambda j=j, r=r, t=pv_ti: emit_pv(j, r, t))
                    )
            for pv_ti in (nt - 2, nt - 1):
                pvv = max(pv_ti - 4 * j, 0)
                wt = w_pv(pvv) + (400 if pv_ti == nt - 1 else 0)
                seq.append((wt, lambda j=j, r=r, t=pv_ti: emit_pv(j, r, t)))
            return seq

        def qk_pieces(j, r, which):
            return [
                (CQK, lambda j=j, r=r, w=which, q=q: emit_qk_quarter(j, r, w, q))
                for q in range(4)
            ]

        def v_pieces(j, ii):
            return [
                (CQK, lambda j=j, ii=ii, q=q: emit_v_quarter(j, ii, q))
                for q in range(4)
            ]

        def proj_pieces(j, sb, f):
            return [
                (CPROJ, lambda j=j, sb=sb, f=f, rr=rr: emit_proj_piece(j, sb, f, rr))
                for rr in ((0, 1), (2, 3))
            ]

        # chunk 0: Q first, ordered so compute starts when the first half
        # of wq and xts[0] has landed, while keeping at most two open
        # accumulation groups (mm512 pool is bufs=2)
        for r0, r1 in ((0, 1), (2, 3)):
            for qi in range(4):
                emit_qk_quarter(0, r0, "q", qi)
                emit_qk_quarter(0, r1, "q", qi)

        norm_carry = None  # deferred norm of the previous pair/chunk
        for j in range(SCH):
            # K(j)/V(j) must precede chunk-j diagonal blocks in program
            # order.  Chunk 0: r0's attention is zipped directly into the
            # K/V stream so ACT starts as soon as K(r0)/V(0..3) land.
            # Chunk 1: K(1)/V(1) already ran as chunk-0 filler, so phase 1
            # is just the non-diag pipeline.  Chunks 2-3: zip K/V into r0's
            # non-diagonal pipeline as before.
            if j == 0:
                for q in range(4):
                    emit_qk_quarter(0, 0, "k", q)
                for q in range(4):
                    emit_qk_quarter(0, 1, "k", q)
                emit_sc(0, 0, 0)
                emit_sc(0, 0, 1)
                for q in range(4):
                    emit_v_quarter(0, 0, q)
                for q in range(4):
                    emit_v_quarter(0, 1, q)
                emit_sc(0, 0, 2)
                emit_sc(0, 0, 3)
                emit_pv(0, 0, 0)
                emit_pv(0, 0, 1)
                for q in range(4):
                    emit_v_quarter(0, 2, q)
                for q in range(4):
                    emit_v_quarter(0, 3, q)
                emit_pv(0, 0, 2)
                emit_pv(0, 0, 3)
                for q in range(4):
                    emit_qk_quarter(0, 2, "k", q)
                for q in range(4):
                    emit_qk_quarter(0, 3, "k", q)
                rest0 = []
            else:
                p0 = pipe(j, 0)
                # entries per sc(ti): 1 for ti<2, else 2 (sc + lagged pv)
                ndlen = 4 * j + max(0, 4 * j - 2)
                nd0, rest0 = p0[:ndlen], p0[ndlen:]
                if norm_carry is not None:
                    nd0.insert(min(3, len(nd0)), norm_carry)
                    norm_carry = None
                kv = []
                for rp in ((0, 1), (2, 3)):
                    for qi in range(4):
                        kv.append(qk_pieces(j, rp[0], "k")[qi])
                        kv.append(qk_pieces(j, rp[1], "k")[qi])
                for ip in ((0, 1), (2, 3)):
                    for qi in range(4):
                        kv.append(v_pieces(j, ip[0])[qi])
                        kv.append(v_pieces(j, ip[1])[qi])
                interleave(nd0, kv)
            # rest: r0 diagonal, then r1..r3; each pair's norm is spliced
            # ~4 entries into the NEXT pair's stream
            main = list(rest0)
            for r in range(1, PAIRS):
                pr_items = pipe(j, r)
                if j == 3 and r == 3:
                    # drop the trailing pv(14)/pv(15): re-emitted below
                    # around the half-norms
                    pr_items = pr_items[:-2]
                pr_items.insert(
                    min(4, len(pr_items)),
                    (CNORM, lambda j=j, r=r - 1: emit_norm(j, r)),
                )
                if j == 3 and r == 2:
                    # after norm(3,1) both r0/r1 cat chunks exist: stash
                    # r0+r1 proj partials for the final s-chunk
                    for u in range(8):
                        pr_items.insert(
                            5 + u, (CPROJ, lambda u=u: emit_proj3_a(u))
                        )
                if j == 3 and r == 3:
                    # fold r2 into the stash so only r3 remains at the tail
                    for u in range(8):
                        pr_items.insert(
                            5 + u, (CPROJ, lambda u=u: emit_proj3_c(u))
                        )
                    # pipelined tail: half-norm 0 + its proj3b units run
                    # before pv(14)/pv(15); then half 1 + the rest
                    pr_items.append((CNORM, lambda: emit_norm33_half(0)))
                    for u in range(4):
                        pr_items.append(
                            (0, lambda u=u: emit_proj3_b(u))
                        )
                    for t in (14, 15):
                        pr_items.append(
                            (0, lambda t=t: emit_pv(3, 3, t))
                        )
                    pr_items.append((0, lambda: emit_norm33_half(1)))
                    for u in range(4, 8):
                        pr_items.append(
                            (0, lambda u=u: emit_proj3_b(u))
                        )
                main += pr_items
            if j + 1 < SCH:
                norm_carry = (CNORM, lambda j=j: emit_norm(j, PAIRS - 1))
            filler = []
            if j + 1 < SCH:
                for rp in ((0, 1), (2, 3)):
                    for qi in range(4):
                        filler.append(qk_pieces(j + 1, rp[0], "q")[qi])
                        filler.append(qk_pieces(j + 1, rp[1], "q")[qi])
            if j == 2:
                for sb in range(0, 6):
                    for f in range(2):
                        filler += proj_pieces(sb // 4, sb, f)
            if j == 3:
                for sb in range(6, 12):
                    for f in range(2):
                        filler += proj_pieces(sb // 4, sb, f)
            interleave(main, filler)

    nc.compile()
    return nc


def get_nc():
    if "nc" not in _CACHE:
        _CACHE["nc"] = _build()
    return _CACHE["nc"]


def prep_core_inputs(x, Wq, bq, Wk, bk, Wv, bv, Wp, core):
    """Pack the full-model inputs into one core's input map."""
    b, g = core // 2, core % 2
    heads = list(range(g * 8, g * 8 + 8))

    def pack_w(W):  # [H,E,D] -> local [E, 512] -> [128, 8, 512]
        Wl = np.concatenate([W[h] for h in heads], axis=1)
        return host_round(Wl.reshape(ET, 128, 512).transpose(1, 0, 2))

    wp_l = host_round(
        Wp[g * 512 : (g + 1) * 512].reshape(PAIRS, 128, E).transpose(1, 0, 2)
    )

    def pack_b(bias):
        return np.stack(
            [
                np.concatenate([bias[heads[2 * r]], bias[heads[2 * r + 1]]])
                for r in range(PAIRS)
            ],
            axis=1,
        ).astype(np.float32)

    bv_cat = np.concatenate([bv[h] for h in heads]).astype(np.float32)

    e2 = np.zeros((128, 128), np.float32)
    e2[64, 0:64] = 1.0
    e2[65, 64:128] = 1.0
    e128 = np.eye(128, dtype=np.float32)

    p = np.arange(128)[:, None, None]
    v = np.arange(4)[None, :, None]
    c = np.arange(512)[None, None, :]
    mask = (c >= p + 128 * v).astype(np.float32)  # [128, 4, 512]
    mask = host_round(np.repeat(mask[:, :, None, :], 2, axis=2))

    xe = x[b].T.reshape(ET, 128, SCH, 512)  # [et, p, j, s]
    return {
        "xt": host_round(np.ascontiguousarray(xe.transpose(2, 1, 0, 3))),
        "wq": pack_w(Wq),
        "wk": pack_w(Wk),
        "wv": pack_w(Wv),
        "wp": wp_l,
        "bq": pack_b(bq),
        "bk": pack_b(bk),
        "bvb": np.tile(bv_cat[None, :], (128, 1)),
        "mask": mask,
        "eye2": host_round(e2),
        "eye128": host_round(e128),
    }


def kernel(**inputs):
    from concourse.bass_utils import run_bass_kernel_spmd

    args = {k: np.asarray(v, np.float32) for k, v in inputs.items()}
    nc = get_nc()
    in_maps = [
        prep_core_inputs(
            args["x"], args["Wq"], args["bq"], args["Wk"], args["bk"],
            args["Wv"], args["bv"], args["Wp"], c,
        )
        for c in range(NCORES)
    ]
    res = run_bass_kernel_spmd(nc, in_maps, core_ids=list(range(NCORES)))
    parts = [np.asarray(r["y"], np.float32) for r in res.results]
    out = np.stack([parts[2 * b] + parts[2 * b + 1] for b in range(B)])
    return (out + args["bp"][None, None, :]).astype(np.float32)
